# revision 44
# baseline (speedup 1.0000x reference)
"""Trainium2 Bass kernel for a pre-LN causal decoder block.

Model: B=4, S=2048, EMBED=1024, HEADS=16, HEAD_DIM=64, FF=4096, fp32 I/O.

Sharding (8 NeuronCores, two SPMD launches):
  Launch 1 (attention): core c -> batch b=c//2, head-half hh=c%2 (8 heads).
    Each core computes LN1, its 512-wide QKV column slice, causal attention
    for its 8 heads, and a partial O-projection. Host sums the two partials
    per batch and adds the residual + folded biases.
  Launch 2 (FFN): tokens (B*S=8192) sharded 8 ways (1024 tokens/core);
    each core runs LN2 + GELU MLP on its tokens with full (folded) W1/W2.

Optimization notes (vs the first working version; ~1013us -> ~556us):
  - AV matmul reoriented (lhsT=probability block, rhs=V strip + ones
    column) so the softmax denominators land as a PSUM *column*;
    normalization is a local per-partition reciprocal+multiply (the old
    DRAM broadcast round-trip is gone).
  - Causal mask matmuls removed entirely: AV consumes only kb <= qt
    blocks, so off-diagonal garbage is never read; the diagonal 128x128
    triangle blocks get a tiny DVE binary-mask multiply instead, and the
    diagonal score matmuls/exps are column-restricted to what is read.
  - Emission is software-pipelined: the next slice's LN/QKV fillers are
    Bresenham-spread between score/exp groups (regions 0-2) and ALL
    O-projections are deferred to region 3 (where exp load peaks), so
    the in-order PE queue never parks while the ACT engine drains exps.
    Normalization trails its AV chain by one q tile so its DVE latency
    hides behind the next chain. PE warm-up transposes ramp the clock
    during the initial x-DMA/LN window.
  - FFN: W1 loaded once (the first 4 chunks stay resident to serve both
    512-token halves while the second half's LN runs), startup DMAs
    reordered (x first, W2 split into late-interleaved chunks).

All matmuls run in bf16 with fp32 PSUM accumulation; LN statistics and
softmax run in fp32. LN affine params and all biases are folded into the
weight matrices / per-channel biases on the host, so the device kernels
implement the fully general module.
"""

import numpy as np
import ml_dtypes

# ---------------------------------------------------------------------------
# Environment patches (in-process only).
# ---------------------------------------------------------------------------


def _install_env():
    import sys
    import types

    # antenv.axon_hooks may be missing in this image; provide it so
    # run_bass_kernel_spmd(trace=True) degrades gracefully / can profile.
    try:
        import antenv.axon_hooks  # noqa: F401
    except ImportError:
        mod = types.ModuleType("antenv.axon_hooks")
        mod._hook = None
        mod.set_axon_ntff_profile_hook = lambda h: setattr(mod, "_hook", h)
        mod.get_axon_ntff_profile_hook = lambda: mod._hook
        sys.modules["antenv.axon_hooks"] = mod
        try:
            import antenv

            antenv.axon_hooks = mod
        except ImportError:
            pass

    import concourse.bass_utils as bu

    # zero-egress sandbox: don't try to copy NEFF dirs to a remote bucket
    bu.upload_artifacts = lambda tmpdir: tmpdir

    # This image's walrus accepts at most ONE sync-wait on a TPB_CTRL
    # (Drain/Nop) instruction; Tile's kernel-tail drain piles every
    # outstanding sem wait onto a single Drain and codegen fails with
    # "Too many sync wait commands". Split the waits across chained
    # single-wait nops (identical semantics: all waits complete on SP
    # before the all-engine barrier / semaphore reset).
    import concourse.mybir as mybir
    import concourse.tile as tile
    from concourse.vector_clock import ScopedClock

    if getattr(tile.TileContext, "_drain_patch_installed", False):
        return

    def _drain_and_barrier(self, tick_clock, wait_clock):
        nc = self.nc
        drain_inst = nc.sync.drain()
        wait_clock.add_sem_waits(
            drain_inst.ins, ScopedClock({None: tick_clock.global_clock})
        )
        si = drain_inst.ins.sync_info
        waits = list(si.on_wait or [])
        if len(waits) > 1:
            si.on_wait = waits[:1]
            for w in waits[1:]:
                nop = nc.sync.nop()
                nop.ins.sync_info = mybir.SyncInfo(on_wait=[w], on_update=[])
        nc.all_engine_barrier()
        assert self.sems is not None
        popped = nc._tile_sem_poison_stack.pop()
        assert popped is self._sem_poison
        nc.clear_and_free_semaphores(list(self.sems.allocated().values()))
        nc.all_engine_barrier()

    tile.TileContext._drain_and_barrier = _drain_and_barrier
    tile.TileContext._drain_patch_installed = True


_install_env()

import concourse.bass as bass  # noqa: E402
from concourse import bacc  # noqa: E402
import concourse.mybir as mybir  # noqa: E402
import concourse.tile as tile  # noqa: E402
from concourse.bass_utils import run_bass_kernel_spmd  # noqa: E402
from concourse.masks import make_identity  # noqa: E402

F32 = mybir.dt.float32
BF16 = mybir.dt.bfloat16
AF = mybir.ActivationFunctionType
OP = mybir.AluOpType
BF16NP = ml_dtypes.bfloat16

B, S, E, H, HD, FF = 4, 2048, 1024, 16, 64, 4096
P = 128
EPS = 1e-5


def _ln_tile(nc, pool_small, x_ap, out_ap, eps_tile):
    """Non-affine LayerNorm of one [128, E] tile; out may be bf16."""
    nsub = E // 512
    stats = pool_small.tile([P, nsub, 6], F32, tag="lnstats")
    for j in range(nsub):
        nc.vector.bn_stats(stats[:, j, :], x_ap[:, j * 512 : (j + 1) * 512])
    mv = pool_small.tile([P, 2], F32, tag="lnmv")
    nc.vector.bn_aggr(mv[:], stats[:])
    rstd = pool_small.tile([P, 1], F32, tag="lnrstd")
    nc.scalar.activation(rstd[:], mv[:, 1:2], AF.Sqrt, bias=eps_tile[:])
    nc.vector.reciprocal(rstd[:], rstd[:])
    nc.vector.tensor_scalar(
        out=out_ap,
        in0=x_ap,
        scalar1=mv[:, 0:1],
        scalar2=rstd[:],
        op0=OP.subtract,
        op1=OP.mult,
    )


def build_attn():
    """Launch 1: per-core attention partial.

    inputs : x[S,E] f32, wq/wk/wv[E,512] bf16, wo[512,E] bf16,
             bq/bk[512] f32, tri[P,2,P] bf16 (lower-triangular 0/1 x2)
    output : out[S,E] bf16  (= y_heads @ wo, partial over head-half;
             the host accumulates the two partials in f32)
    """
    nc = bacc.Bacc("TRN2", target_bir_lowering=False, debug=False, num_devices=8)
    x_d = nc.dram_tensor("x", [S, E], F32, kind="ExternalInput")
    wq_d = nc.dram_tensor("wq", [E, 512], BF16, kind="ExternalInput")
    wk_d = nc.dram_tensor("wk", [E, 512], BF16, kind="ExternalInput")
    wv_d = nc.dram_tensor("wv", [E, 512], BF16, kind="ExternalInput")
    wo_d = nc.dram_tensor("wo", [512, E], BF16, kind="ExternalInput")
    bq_d = nc.dram_tensor("bq", [512], F32, kind="ExternalInput")
    bk_d = nc.dram_tensor("bk", [512], F32, kind="ExternalInput")
    tri_d = nc.dram_tensor("tri", [P, 2, P], BF16, kind="ExternalInput")
    out_d = nc.dram_tensor("out", [S, E], BF16, kind="ExternalOutput")

    NT = S // P  # 16 token tiles
    NQ = S // 512  # 4 q slices

    with tile.TileContext(nc) as tc:
        with (
            tc.tile_pool(name="consts", bufs=1) as consts,
            tc.tile_pool(name="state", bufs=1) as state,
            tc.tile_pool(name="xin", bufs=4) as xin,
            tc.tile_pool(name="hp", bufs=2) as hpool,
            tc.tile_pool(name="hT", bufs=2) as hTpool,
            tc.tile_pool(name="qT", bufs=2) as qTpool,
            tc.tile_pool(name="pt", bufs=2) as ptpool,
            tc.tile_pool(name="ptm", bufs=3) as ptmpool,
            tc.tile_pool(name="yq", bufs=3) as yqpool,
            tc.tile_pool(name="yT", bufs=4) as yTpool,
            tc.tile_pool(name="sums", bufs=4) as sumspool,
            tc.tile_pool(name="small", bufs=6) as small,
            tc.tile_pool(name="psS", bufs=2, space="PSUM") as psS,
            tc.tile_pool(name="psA", bufs=2, space="PSUM") as psA,
            tc.tile_pool(name="psM", bufs=2, space="PSUM") as psM,
        ):
            ident = consts.tile([P, P], BF16)
            eps_t = consts.tile([P, 1], F32)
            tri_sb = consts.tile([P, 2, P], BF16)
            wq_sb = consts.tile([P, 8, 512], BF16)
            wk_sb = consts.tile([P, 8, 512], BF16)
            wv_sb = consts.tile([P, 8, 512], BF16)
            wo_sb = consts.tile([P, 4, E], BF16)
            bq_sb = consts.tile([P, 4], F32)
            bk_sb = consts.tile([P, 4], F32)

            kTs = [state.tile([P, S], BF16, name=f"kT{i}") for i in range(4)]
            v_sb = state.tile([P, NT, 8 * 65], BF16)  # [t_in, t_chunk, strip]

            # --- preamble: first x tiles ahead of everything, then weights
            xts = {}
            wvr = wv_d.rearrange("(eo p) c -> p eo c", p=P)
            wqr = wq_d.rearrange("(eo p) c -> p eo c", p=P)
            wkr = wk_d.rearrange("(eo p) c -> p eo c", p=P)
            for ti in range(2):
                xt = xin.tile([P, E], F32, tag="xt", name=f"xt{ti}")
                nc.sync.dma_start(xt[:], x_d[ti * P : (ti + 1) * P, :])
                xts[ti] = xt
            for g in range(2):
                sl = slice(g * 4, (g + 1) * 4)
                nc.sync.dma_start(wv_sb[:, sl, :], wvr[:, sl, :])
            for ti in range(2, 4):
                xt = xin.tile([P, E], F32, tag="xt", name=f"xt{ti}")
                nc.sync.dma_start(xt[:], x_d[ti * P : (ti + 1) * P, :])
                xts[ti] = xt
            nc.sync.dma_start(tri_sb[:], tri_d[:])
            make_identity(nc, ident)
            nc.vector.memset(eps_t[:], EPS)
            nc.vector.memset(
                v_sb[:].rearrange("p t (h c) -> p t h c", c=65)[:, :, :, 64:65], 1.0
            )
            for g in range(2):
                sl = slice(g * 4, (g + 1) * 4)
                nc.sync.dma_start(wq_sb[:, sl, :], wqr[:, sl, :])
                nc.sync.dma_start(wk_sb[:, sl, :], wkr[:, sl, :])
            nc.sync.dma_start(bq_sb[:], bq_d.rearrange("(cc p) -> p cc", p=P))
            nc.sync.dma_start(bk_sb[:], bk_d.rearrange("(cc p) -> p cc", p=P))
            wor = wo_d.rearrange("(co p) e -> p co e", p=P)
            for g in range(2):
                sl = slice(g * 2, (g + 1) * 2)
                nc.sync.dma_start(wo_sb[:, sl, :], wor[:, sl, :])

            # PE warm-up: dummy transposes ramp the tensor-engine
            # p-state while the first x tiles and LN are still in flight.
            psW = psA.tile([P, P], BF16, tag="a", name="psW")
            for _ in range(80):
                nc.tensor.transpose(psW[:], ident[:], ident)

            # per-tsl live tiles
            hTs = {}  # tsl -> [P, 8, 512]
            qts = {}  # tsl -> [P, 4, 512]
            pts = {}  # (tsl, hc) -> [P, NT, 2, 512]
            yTs = {}  # tsl -> [P, 4, 512]

            # ---------------- chunk emitters ----------------
            def A_ti(tsl, ti):
                """LN + transpose + V-projection for token tile ti."""
                loc = (ti - tsl * 4) * P
                if ti in xts:
                    xt = xts.pop(ti)
                else:
                    xt = xin.tile([P, E], F32, tag="xt")
                    nc.sync.dma_start(xt[:], x_d[ti * P : (ti + 1) * P, :])
                ht = hpool.tile([P, E], BF16)
                _ln_tile(nc, small, xt[:], ht[:], eps_t)
                hT = hTs[tsl]
                for g in range(2):
                    trp = psA.tile([P, 4, P], BF16, tag="a")
                    for j in range(4):
                        ec = g * 4 + j
                        nc.tensor.transpose(
                            trp[:, j, :], ht[:, ec * P : (ec + 1) * P], ident
                        )
                    nc.vector.tensor_copy(
                        hT[:, g * 4 : (g + 1) * 4, loc : loc + P], trp[:]
                    )
                psv = psA.tile([P, 512], F32, tag="a")
                for ec in range(8):
                    nc.tensor.matmul(
                        psv[:],
                        lhsT=hT[:, ec, loc : loc + P],
                        rhs=wv_sb[:, ec, :],
                        start=(ec == 0),
                        stop=(ec == 7),
                    )
                nc.vector.tensor_copy(
                    v_sb[:, ti, :].rearrange("p (h c) -> p h c", c=65)[:, :, 0:64],
                    psv[:].rearrange("p (h c) -> p h c", c=64),
                )

            def A_qk(tsl, cc):
                """Q and K projection for c-chunk cc of slice tsl."""
                hT = hTs[tsl]
                psq = psA.tile([P, 512], F32, tag="a")
                psk = psA.tile([P, 512], F32, tag="a")
                for ec in range(8):
                    nc.tensor.matmul(
                        psq[:],
                        lhsT=wq_sb[:, ec, cc * P : (cc + 1) * P],
                        rhs=hT[:, ec, :],
                        start=(ec == 0),
                        stop=(ec == 7),
                    )
                    nc.tensor.matmul(
                        psk[:],
                        lhsT=wk_sb[:, ec, cc * P : (cc + 1) * P],
                        rhs=hT[:, ec, :],
                        start=(ec == 0),
                        stop=(ec == 7),
                    )
                nc.vector.tensor_scalar(
                    out=qts[tsl][:, cc, :],
                    in0=psq[:],
                    scalar1=bq_sb[:, cc : cc + 1],
                    scalar2=None,
                    op0=OP.add,
                )
                nc.vector.tensor_scalar(
                    out=kTs[cc][:, tsl * 512 : (tsl + 1) * 512],
                    in0=psk[:],
                    scalar1=bk_sb[:, cc : cc + 1],
                    scalar2=None,
                    op0=OP.add,
                )

            def B_sc(tsl, hc, par, g):
                """Scores + exp for kb pair g, head parity par of pair hc.

                For diagonal kb blocks (kb >= 4*tsl) only q columns at or
                beyond the block's diagonal offset are ever consumed by
                B_av, so the matmul and exp are restricted accordingly.
                """
                kT = kTs[hc]
                psX = psS.tile([P, 2, 512], F32, tag="s")
                for j in range(2):
                    kb = g * 2 + j
                    off = max(0, kb - 4 * tsl) * P
                    ksl = slice(kb * P, (kb + 1) * P)
                    nc.tensor.matmul(
                        psX[:, j, off:512],
                        lhsT=kT[par * 64 : (par + 1) * 64, ksl],
                        rhs=qts[tsl][par * 64 : (par + 1) * 64, hc, off:512],
                        start=True,
                        stop=True,
                    )
                goff = max(0, g * 2 - 4 * tsl) * P
                nc.scalar.activation(
                    pts[(tsl, hc)][:, g * 2 : (g + 1) * 2, par, goff:512],
                    psX[:, :, goff:512],
                    AF.Exp,
                    scale=0.125,
                )

            def AVq(tsl, hc, qt):
                """AV accumulation for q tile qt; returns its PSUM tile."""
                loc = (qt - tsl * 4) * P
                pt = pts[(tsl, hc)]
                ptm = ptmpool.tile([P, 2, P], BF16, tag="ptm")
                nc.vector.tensor_tensor(
                    out=ptm[:],
                    in0=pt[:, qt, :, loc : loc + P],
                    in1=tri_sb[:],
                    op=OP.mult,
                )
                psy = psM.tile([P, 2, 65], F32, tag="m")
                for par in range(2):
                    strip = slice((2 * hc + par) * 65, (2 * hc + par) * 65 + 65)
                    for kb in range(qt + 1):
                        lhsT = (
                            ptm[:, par, :]
                            if kb == qt
                            else pt[:, kb, par, loc : loc + P]
                        )
                        nc.tensor.matmul(
                            psy[:, par, :],
                            lhsT=lhsT,
                            rhs=v_sb[:, kb, strip],
                            start=(kb == 0),
                            stop=(kb == qt),
                        )
                return psy

            def NORMq(tsl, hc, qt, psy):
                """Normalize + transpose q tile qt into yT (c-major).

                Emitted one q tile behind AVq so the ytr transpose's wait
                on the DVE normalize hides behind the next AV chain.
                """
                loc = (qt - tsl * 4) * P
                rec = sumspool.tile([P, 2, 1], F32, tag="rec")
                nc.vector.reciprocal(rec[:], psy[:, :, 64:65])
                yqt = yqpool.tile([P, P], BF16)
                for par in range(2):
                    nc.vector.tensor_scalar(
                        out=yqt[:, par * 64 : (par + 1) * 64],
                        in0=psy[:, par, 0:64],
                        scalar1=rec[:, par, :],
                        scalar2=None,
                        op0=OP.mult,
                    )
                psT = psA.tile([P, P], BF16, tag="a")
                nc.tensor.transpose(psT[:], yqt[:], ident)
                nc.vector.tensor_copy(yTs[tsl][:, hc, loc : loc + P], psT[:])

            def B_av(tsl, hc):
                """AV + normalize for all 4 q tiles of head pair hc."""
                pend = []
                for q in range(tsl * 4, tsl * 4 + 4):
                    psy = AVq(tsl, hc, q)
                    pend.append((q, psy))
                    if len(pend) > 1:
                        q0, psy0 = pend.pop(0)
                        NORMq(tsl, hc, q0, psy0)
                while pend:
                    q0, psy0 = pend.pop(0)
                    NORMq(tsl, hc, q0, psy0)

            def O_ti(tsl, ti, eg):
                """O-projection for token tile ti, embed half eg."""
                loc = (ti - tsl * 4) * P
                yT = yTs[tsl]
                pso = psA.tile([P, 512], F32, tag="a")
                for cc in range(4):
                    nc.tensor.matmul(
                        pso[:],
                        lhsT=yT[:, cc, loc : loc + P],
                        rhs=wo_sb[:, cc, eg * 512 : (eg + 1) * 512],
                        start=(cc == 0),
                        stop=(cc == 3),
                    )
                ot = yqpool.tile([P, 512], BF16, tag="ot")
                nc.vector.tensor_copy(ot[:], pso[:])
                nc.sync.dma_start(
                    out_d[ti * P : (ti + 1) * P, eg * 512 : (eg + 1) * 512], ot[:]
                )

            # ---------------- schedule ----------------
            def prep_slice(tsl):
                hTs[tsl] = hTpool.tile([P, 8, 512], BF16, tag="hT", name=f"hT{tsl}")
                qts[tsl] = qTpool.tile([P, 4, 512], BF16, tag="qT", name=f"qT{tsl}")
                yTs[tsl] = yTpool.tile([P, 4, 512], BF16, tag="yT", name=f"yT{tsl}")

            prep_slice(0)
            for ti in range(4):
                A_ti(0, ti)
            for cc in range(4):
                A_qk(0, cc)

            for tsl in range(NQ):
                nkb = 4 * tsl + 4
                # filler chunks emitted between score groups so the PE
                # never parks while ACT drains exps: next slice's LN/QKV
                # in regions 0-2; ALL deferred O-projections in region 3
                # (the most ACT-bound region, where exp load peaks).
                fillers = []
                if tsl + 1 < NQ:
                    prep_slice(tsl + 1)
                    for ti in range((tsl + 1) * 4, (tsl + 1) * 4 + 4):
                        xt = xin.tile([P, E], F32, tag="xt", name=f"xt{ti}")
                        nc.sync.dma_start(xt[:], x_d[ti * P : (ti + 1) * P, :])
                        xts[ti] = xt
                        fillers.append(("A", tsl + 1, ti, None))
                    for cc in range(4):
                        fillers.append(("Q", tsl + 1, cc, None))
                else:
                    for ts0 in range(3):
                        for ti in range(ts0 * 4, ts0 * 4 + 4):
                            for eg in range(2):
                                fillers.append(("O", ts0, ti, eg))

                def emit_filler():
                    if not fillers:
                        return
                    kind, a, b, c = fillers.pop(0)
                    if kind == "O":
                        O_ti(a, b, c)
                    elif kind == "A":
                        A_ti(a, b)
                    else:
                        A_qk(a, b)

                # score groups per (hc, par): nkb//2 of them; spread the
                # fillers evenly across them (Bresenham) so the PE always
                # has non-score work between groups while ACT drains exps.
                ngroups = 4 * 2 * (nkb // 2)
                nfill = len(fillers)
                gcount = 0
                npop = 0
                for hc in range(4):
                    pts[(tsl, hc)] = ptpool.tile(
                        [P, nkb, 2, 512], BF16, tag="pt", name=f"pt{tsl}_{hc}"
                    )
                    for par in range(2):
                        for g in range(nkb // 2):
                            B_sc(tsl, hc, par, g)
                            gcount += 1
                            while npop * ngroups < gcount * nfill:
                                emit_filler()
                                npop += 1
                    if hc >= 1:
                        B_av(tsl, hc - 1)
                pend3 = []
                normed = []

                def pop_norm():
                    q0, psy0 = pend3.pop(0)
                    NORMq(tsl, 3, q0, psy0)
                    normed.append(q0)

                def pop_o():
                    # O trails NORM by one q tile so its wait on the yT
                    # copy hides behind the next AV/NORM work
                    q0 = normed.pop(0)
                    for eg in range(2):
                        O_ti(tsl, q0, eg)

                for qt in range(tsl * 4, tsl * 4 + 4):
                    psy = AVq(tsl, 3, qt)
                    pend3.append((qt, psy))
                    if len(pend3) > 1:
                        pop_norm()
                        if tsl == NQ - 1 and len(normed) > 1:
                            pop_o()
                    emit_filler()
                while pend3:
                    pop_norm()
                    if tsl == NQ - 1 and len(normed) > 1:
                        pop_o()
                if tsl == NQ - 1:
                    while normed:
                        pop_o()
                while fillers:
                    emit_filler()
    nc.compile()
    return nc


def build_ffn():
    """Launch 2: LN2 + GELU MLP + residual on a 1024-token slice.

    inputs : x2[1024,E] f32, w1[E,FF] bf16, w2[FF,E] bf16, b1[FF] f32
    output : out[1024,E] f32  (= x2 + gelu(LN(x2) @ w1 + b1) @ w2)

    ff1 is computed transposed (gT[f,t]) so the gelu output feeds the second
    matmul as lhsT without a transpose. W1 is loaded once; each chunk serves
    both 512-token halves. W2 arrives in chunks interleaved late (it is not
    needed until phase C), so startup DMA bandwidth goes to x and W1.
    """
    T = 1024
    nc = bacc.Bacc("TRN2", target_bir_lowering=False, debug=False, num_devices=8)
    x2_d = nc.dram_tensor("x2", [T, E], F32, kind="ExternalInput")
    w1_d = nc.dram_tensor("w1", [E, FF], BF16, kind="ExternalInput")
    w2_d = nc.dram_tensor("w2", [FF, E], BF16, kind="ExternalInput")
    b1_d = nc.dram_tensor("b1", [FF], F32, kind="ExternalInput")
    out_d = nc.dram_tensor("out", [T, E], F32, kind="ExternalOutput")

    NT = T // P  # 8 token tiles
    NF = FF // P  # 32 f chunks
    NFG = FF // 256  # 16 w1 dma chunks

    with tile.TileContext(nc) as tc:
        with (
            tc.tile_pool(name="consts", bufs=1) as consts,
            tc.tile_pool(name="state", bufs=1) as state,
            tc.tile_pool(name="w1p", bufs=6) as w1pool,
            tc.tile_pool(name="xin", bufs=4) as xin,
            tc.tile_pool(name="hp", bufs=2) as hpool,
            tc.tile_pool(name="outp", bufs=3) as outp,
            tc.tile_pool(name="small", bufs=6) as small,
            tc.tile_pool(name="psB", bufs=2, space="PSUM") as psB,
            tc.tile_pool(name="psC", bufs=4, space="PSUM") as psC,
        ):
            ident = consts.tile([P, P], BF16)
            eps_t = consts.tile([P, 1], F32)
            w2_sb = consts.tile([P, NF, E], BF16)
            b1_sb = consts.tile([P, NF], F32)
            h2T = state.tile([P, 8, T], BF16)  # [e_in, e_chunk, t]
            gT = state.tile([P, NF, T], BF16)  # [f_in, f_chunk, t]

            # preamble: x tiles first, then identity/eps/b1; w2 is emitted
            # in chunks interleaved into the ff1 loop below.
            xts = {}

            def issue_x(ti):
                xt = xin.tile([P, E], F32, tag="xt", name=f"xt{ti}")
                nc.sync.dma_start(xt[:], x2_d[ti * P : (ti + 1) * P, :])
                xts[ti] = xt

            for ti in range(4):
                issue_x(ti)
            make_identity(nc, ident)
            nc.vector.memset(eps_t[:], EPS)
            nc.sync.dma_start(b1_sb[:], b1_d.rearrange("(fo p) -> p fo", p=P))
            w1r = w1_d.rearrange("(eo p) f -> p eo f", p=P)
            w2r = w2_d.rearrange("(fo p) e -> p fo e", p=P)

            w1g_tiles = {}

            def issue_w1(fg):
                t = w1pool.tile([P, 8, 256], BF16, tag="w1")
                nc.sync.dma_start(t[:], w1r[:, :, fg * 256 : (fg + 1) * 256])
                w1g_tiles[fg] = t

            issue_w1(0)
            issue_w1(1)

            # PE warm-up: ramp the tensor-engine p-state while the first
            # x tiles and LN are still in flight.
            psW = psB.tile([P, P], BF16, tag="u", name="psW")
            for _ in range(60):
                nc.tensor.transpose(psW[:], ident[:], ident)

            def A_ti(to):
                xt = xts.pop(to)
                h2 = hpool.tile([P, E], BF16)
                _ln_tile(nc, small, xt[:], h2[:], eps_t)
                for g in range(2):
                    trp = psB.tile([P, 4, P], BF16, tag="u")
                    for j in range(4):
                        ec = g * 4 + j
                        nc.tensor.transpose(
                            trp[:, j, :], h2[:, ec * P : (ec + 1) * P], ident
                        )
                    nc.vector.tensor_copy(
                        h2T[:, g * 4 : (g + 1) * 4, to * P : (to + 1) * P],
                        trp[:],
                    )

            def ff1(fg, tsl, w1g):
                tofs = tsl * 512
                ps0 = psB.tile([P, 2, 512], F32, tag="u")
                for ec in range(8):
                    for j in range(2):
                        nc.tensor.matmul(
                            ps0[:, j, :],
                            lhsT=w1g[:, ec, j * P : (j + 1) * P],
                            rhs=h2T[:, ec, tofs : tofs + 512],
                            start=(ec == 0),
                            stop=(ec == 7),
                        )
                for j in range(2):
                    fc = fg * 2 + j
                    nc.scalar.activation(
                        gT[:, fc, tofs : tofs + 512],
                        ps0[:, j, :],
                        AF.Gelu,
                        bias=b1_sb[:, fc : fc + 1],
                    )

            def issue_w2(fg):
                if fg % 2 == 0 and fg // 2 < 8:
                    wsl = slice((fg // 2) * 4, (fg // 2) * 4 + 4)
                    nc.sync.dma_start(w2_sb[:, wsl, :], w2r[:, wsl, :])

            # ---- Phase A/B interleaved: LN+transpose first 4 tiles, then
            # ff1 on the first half while the second half's LN runs; the
            # first 4 W1 chunks stay resident so their tsl=1 pass follows.
            for to in range(4):
                A_ti(to)
            held = {}
            for fg in range(4):
                w1g = w1g_tiles.pop(fg)
                if fg + 2 < NFG:
                    issue_w1(fg + 2)
                issue_w2(fg)
                issue_x(4 + fg)
                ff1(fg, 0, w1g)
                A_ti(4 + fg)
                held[fg] = w1g
            for fg in range(4):
                ff1(fg, 1, held.pop(fg))
            for fg in range(4, NFG):
                w1g = w1g_tiles.pop(fg)
                if fg + 2 < NFG:
                    issue_w1(fg + 2)
                issue_w2(fg)
                ff1(fg, 0, w1g)
                ff1(fg, 1, w1g)

            # ---- Phase C: out = x2 + gT^T @ W2 ----
            for tb in range(NT):
                psO = psC.tile([P, 512], F32, tag="c")
                psP = psC.tile([P, 512], F32, tag="c")
                for fc in range(NF):
                    nc.tensor.matmul(
                        psO[:],
                        lhsT=gT[:, fc, tb * P : (tb + 1) * P],
                        rhs=w2_sb[:, fc, 0:512],
                        start=(fc == 0),
                        stop=(fc == NF - 1),
                    )
                    nc.tensor.matmul(
                        psP[:],
                        lhsT=gT[:, fc, tb * P : (tb + 1) * P],
                        rhs=w2_sb[:, fc, 512:1024],
                        start=(fc == 0),
                        stop=(fc == NF - 1),
                    )
                xr = xin.tile([P, E], F32, tag="xt")
                nc.sync.dma_start(xr[:], x2_d[tb * P : (tb + 1) * P, :])
                for eg, psX in ((0, psO), (1, psP)):
                    ot = outp.tile([P, 512], F32)
                    nc.vector.tensor_tensor(
                        out=ot[:],
                        in0=psX[:],
                        in1=xr[:, eg * 512 : (eg + 1) * 512],
                        op=OP.add,
                    )
                    nc.sync.dma_start(
                        out_d[tb * P : (tb + 1) * P, eg * 512 : (eg + 1) * 512],
                        ot[:],
                    )
    nc.compile()
    return nc


# ---------------------------------------------------------------------------
# Host orchestration
# ---------------------------------------------------------------------------


def _bf16(a):
    return np.ascontiguousarray(np.asarray(a, dtype=np.float32)).astype(BF16NP)


def _f32(a):
    return np.ascontiguousarray(np.asarray(a, dtype=np.float32))


def _tri01():
    kp = np.arange(P)[:, None]
    qf = np.arange(P)[None, :]
    t = (kp <= qf).astype(np.float32)
    return np.ascontiguousarray(np.stack([t, t], axis=1)).astype(BF16NP)


def kernel(
    x, Wq, bq, Wk, bk, Wv, bv, Wo, bo, g1, beta1, g2, beta2, W1, b1, W2, b2
):
    out, _ = _run(
        x, Wq, bq, Wk, bk, Wv, bv, Wo, bo, g1, beta1, g2, beta2, W1, b1, W2, b2
    )
    return out


def _run(
    x, Wq, bq, Wk, bk, Wv, bv, Wo, bo, g1, beta1, g2, beta2, W1, b1, W2, b2,
    trace=False,
):
    x = _f32(x)
    Wq, bq = _f32(Wq), _f32(bq)
    Wk, bk = _f32(Wk), _f32(bk)
    Wv, bv = _f32(Wv), _f32(bv)
    Wo, bo = _f32(Wo), _f32(bo)
    g1, beta1 = _f32(g1), _f32(beta1)
    g2, beta2 = _f32(g2), _f32(beta2)
    W1, b1 = _f32(W1), _f32(b1)
    W2, b2 = _f32(W2), _f32(b2)

    # Fold LN1 affine into the QKV projections: h = ln0*g1+beta1 =>
    # h@W + b == ln0@(g1[:,None]*W) + (beta1@W + b)
    Wq_e, bq_e = Wq * g1[:, None], beta1 @ Wq + bq
    Wk_e, bk_e = Wk * g1[:, None], beta1 @ Wk + bk
    Wv_e, bv_e = Wv * g1[:, None], beta1 @ Wv + bv
    # V-bias rides through the attention average (rows of attn sum to 1):
    # y = P@(v + bv) = P@v + bv  =>  fold bv@Wo into the residual bias.
    bo_e = bo + bv_e @ Wo
    # Fold LN2 affine into W1.
    W1_e, b1_e = W1 * g2[:, None], beta2 @ W1 + b1

    tri = _tri01()
    nc1 = build_attn()
    in_maps1 = []
    for c in range(8):
        b_, hh = c // 2, c % 2
        cs = 512 * hh
        in_maps1.append(
            {
                "x": x[b_],
                "wq": _bf16(Wq_e[:, cs : cs + 512]),
                "wk": _bf16(Wk_e[:, cs : cs + 512]),
                "wv": _bf16(Wv_e[:, cs : cs + 512]),
                "wo": _bf16(Wo[cs : cs + 512, :]),
                "bq": bq_e[cs : cs + 512],
                "bk": bk_e[cs : cs + 512],
                "tri": tri,
            }
        )
    res1 = run_bass_kernel_spmd(nc1, in_maps1, list(range(8)), trace=trace)
    x2 = x + bo_e[None, None, :]
    for c in range(8):
        x2[c // 2] += np.asarray(res1.results[c]["out"], dtype=np.float32)

    x2f = np.ascontiguousarray(x2.reshape(B * S, E), dtype=np.float32)
    w1b, w2b = _bf16(W1_e), _bf16(W2)
    nc2 = build_ffn()
    in_maps2 = [
        {
            "x2": x2f[c * 1024 : (c + 1) * 1024],
            "w1": w1b,
            "w2": w2b,
            "b1": b1_e,
        }
        for c in range(8)
    ]
    res2 = run_bass_kernel_spmd(nc2, in_maps2, list(range(8)), trace=trace)
    out = np.concatenate([res2.results[c]["out"] for c in range(8)], axis=0)
    out = out + b2[None, :]
    times = (res1.exec_time_ns, res2.exec_time_ns)
    return out.reshape(B, S, E).astype(np.float32), times


# revision 45
# speedup vs baseline: 1.0000x; 1.0000x over previous
"""Trainium2 Bass kernel for a pre-LN causal decoder block.

Model: B=4, S=2048, EMBED=1024, HEADS=16, HEAD_DIM=64, FF=4096, fp32 I/O.

Sharding (8 NeuronCores, two SPMD launches):
  Launch 1 (attention): core c -> batch b=c//2, head-half hh=c%2 (8 heads).
    Each core computes LN1, its 512-wide QKV column slice, causal attention
    for its 8 heads, and a partial O-projection. Host sums the two partials
    per batch and adds the residual + folded biases.
  Launch 2 (FFN): tokens (B*S=8192) sharded 8 ways (1024 tokens/core);
    each core runs LN2 + GELU MLP on its tokens with full (folded) W1/W2.

Optimization notes (vs the first working version; ~1013us -> ~556us):
  - AV matmul reoriented (lhsT=probability block, rhs=V strip + ones
    column) so the softmax denominators land as a PSUM *column*;
    normalization is a local per-partition reciprocal+multiply (the old
    DRAM broadcast round-trip is gone).
  - Causal mask matmuls removed entirely: AV consumes only kb <= qt
    blocks, so off-diagonal garbage is never read; the diagonal 128x128
    triangle blocks get a tiny DVE binary-mask multiply instead, and the
    diagonal score matmuls/exps are column-restricted to what is read.
  - Emission is software-pipelined: the next slice's LN/QKV fillers are
    Bresenham-spread between score/exp groups (regions 0-2) and ALL
    O-projections are deferred to region 3 (where exp load peaks), so
    the in-order PE queue never parks while the ACT engine drains exps.
    Normalization trails its AV chain by one q tile so its DVE latency
    hides behind the next chain. PE warm-up transposes ramp the clock
    during the initial x-DMA/LN window.
  - FFN: W1 loaded once (the first 4 chunks stay resident to serve both
    512-token halves while the second half's LN runs), startup DMAs
    reordered (x first, W2 split into late-interleaved chunks).

All matmuls run in bf16 with fp32 PSUM accumulation; LN statistics and
softmax run in fp32. LN affine params and all biases are folded into the
weight matrices / per-channel biases on the host, so the device kernels
implement the fully general module.
"""

import numpy as np
import ml_dtypes

# ---------------------------------------------------------------------------
# Environment patches (in-process only).
# ---------------------------------------------------------------------------


def _install_env():
    import sys
    import types

    # antenv.axon_hooks may be missing in this image; provide it so
    # run_bass_kernel_spmd(trace=True) degrades gracefully / can profile.
    try:
        import antenv.axon_hooks  # noqa: F401
    except ImportError:
        mod = types.ModuleType("antenv.axon_hooks")
        mod._hook = None
        mod.set_axon_ntff_profile_hook = lambda h: setattr(mod, "_hook", h)
        mod.get_axon_ntff_profile_hook = lambda: mod._hook
        sys.modules["antenv.axon_hooks"] = mod
        try:
            import antenv

            antenv.axon_hooks = mod
        except ImportError:
            pass

    import concourse.bass_utils as bu

    # zero-egress sandbox: don't try to copy NEFF dirs to a remote bucket
    bu.upload_artifacts = lambda tmpdir: tmpdir

    # This image's walrus accepts at most ONE sync-wait on a TPB_CTRL
    # (Drain/Nop) instruction; Tile's kernel-tail drain piles every
    # outstanding sem wait onto a single Drain and codegen fails with
    # "Too many sync wait commands". Split the waits across chained
    # single-wait nops (identical semantics: all waits complete on SP
    # before the all-engine barrier / semaphore reset).
    import concourse.mybir as mybir
    import concourse.tile as tile
    from concourse.vector_clock import ScopedClock

    if getattr(tile.TileContext, "_drain_patch_installed", False):
        return

    def _drain_and_barrier(self, tick_clock, wait_clock):
        nc = self.nc
        drain_inst = nc.sync.drain()
        wait_clock.add_sem_waits(
            drain_inst.ins, ScopedClock({None: tick_clock.global_clock})
        )
        si = drain_inst.ins.sync_info
        waits = list(si.on_wait or [])
        if len(waits) > 1:
            si.on_wait = waits[:1]
            for w in waits[1:]:
                nop = nc.sync.nop()
                nop.ins.sync_info = mybir.SyncInfo(on_wait=[w], on_update=[])
        nc.all_engine_barrier()
        assert self.sems is not None
        popped = nc._tile_sem_poison_stack.pop()
        assert popped is self._sem_poison
        nc.clear_and_free_semaphores(list(self.sems.allocated().values()))
        nc.all_engine_barrier()

    tile.TileContext._drain_and_barrier = _drain_and_barrier
    tile.TileContext._drain_patch_installed = True


_install_env()

import concourse.bass as bass  # noqa: E402
from concourse import bacc  # noqa: E402
import concourse.mybir as mybir  # noqa: E402
import concourse.tile as tile  # noqa: E402
from concourse.bass_utils import run_bass_kernel_spmd  # noqa: E402
from concourse.masks import make_identity  # noqa: E402

F32 = mybir.dt.float32
BF16 = mybir.dt.bfloat16
AF = mybir.ActivationFunctionType
OP = mybir.AluOpType
BF16NP = ml_dtypes.bfloat16

B, S, E, H, HD, FF = 4, 2048, 1024, 16, 64, 4096
P = 128
EPS = 1e-5


def _ln_tile(nc, pool_small, x_ap, out_ap, eps_tile):
    """Non-affine LayerNorm of one [128, E] tile; out may be bf16."""
    nsub = E // 512
    stats = pool_small.tile([P, nsub, 6], F32, tag="lnstats")
    for j in range(nsub):
        nc.vector.bn_stats(stats[:, j, :], x_ap[:, j * 512 : (j + 1) * 512])
    mv = pool_small.tile([P, 2], F32, tag="lnmv")
    nc.vector.bn_aggr(mv[:], stats[:])
    rstd = pool_small.tile([P, 1], F32, tag="lnrstd")
    nc.scalar.activation(rstd[:], mv[:, 1:2], AF.Sqrt, bias=eps_tile[:])
    nc.vector.reciprocal(rstd[:], rstd[:])
    nc.vector.tensor_scalar(
        out=out_ap,
        in0=x_ap,
        scalar1=mv[:, 0:1],
        scalar2=rstd[:],
        op0=OP.subtract,
        op1=OP.mult,
    )


def build_attn():
    """Launch 1: per-core attention partial.

    inputs : x[S,E] f32, wq/wk/wv[E,512] bf16, wo[512,E] bf16,
             bq/bk[512] f32, tri[P,2,P] bf16 (lower-triangular 0/1 x2)
    output : out[S,E] bf16  (= y_heads @ wo, partial over head-half;
             the host accumulates the two partials in f32)
    """
    nc = bacc.Bacc("TRN2", target_bir_lowering=False, debug=False, num_devices=8)
    x_d = nc.dram_tensor("x", [S, E], F32, kind="ExternalInput")
    wq_d = nc.dram_tensor("wq", [E, 512], BF16, kind="ExternalInput")
    wk_d = nc.dram_tensor("wk", [E, 512], BF16, kind="ExternalInput")
    wv_d = nc.dram_tensor("wv", [E, 512], BF16, kind="ExternalInput")
    wo_d = nc.dram_tensor("wo", [512, E], BF16, kind="ExternalInput")
    bq_d = nc.dram_tensor("bq", [512], F32, kind="ExternalInput")
    bk_d = nc.dram_tensor("bk", [512], F32, kind="ExternalInput")
    tri_d = nc.dram_tensor("tri", [P, 2, P], BF16, kind="ExternalInput")
    out_d = nc.dram_tensor("out", [S, E], BF16, kind="ExternalOutput")

    NT = S // P  # 16 token tiles
    NQ = S // 512  # 4 q slices

    with tile.TileContext(nc) as tc:
        with (
            tc.tile_pool(name="consts", bufs=1) as consts,
            tc.tile_pool(name="state", bufs=1) as state,
            tc.tile_pool(name="xin", bufs=4) as xin,
            tc.tile_pool(name="hp", bufs=2) as hpool,
            tc.tile_pool(name="hT", bufs=2) as hTpool,
            tc.tile_pool(name="qT", bufs=2) as qTpool,
            tc.tile_pool(name="pt", bufs=2) as ptpool,
            tc.tile_pool(name="ptm", bufs=3) as ptmpool,
            tc.tile_pool(name="yq", bufs=3) as yqpool,
            tc.tile_pool(name="yT", bufs=4) as yTpool,
            tc.tile_pool(name="sums", bufs=4) as sumspool,
            tc.tile_pool(name="small", bufs=6) as small,
            tc.tile_pool(name="psS", bufs=2, space="PSUM") as psS,
            tc.tile_pool(name="psA", bufs=2, space="PSUM") as psA,
            tc.tile_pool(name="psM", bufs=2, space="PSUM") as psM,
        ):
            ident = consts.tile([P, P], BF16)
            eps_t = consts.tile([P, 1], F32)
            tri_sb = consts.tile([P, 2, P], BF16)
            wq_sb = consts.tile([P, 8, 512], BF16)
            wk_sb = consts.tile([P, 8, 512], BF16)
            wv_sb = consts.tile([P, 8, 512], BF16)
            wo_sb = consts.tile([P, 4, E], BF16)
            bq_sb = consts.tile([P, 4], F32)
            bk_sb = consts.tile([P, 4], F32)

            kTs = [state.tile([P, S], BF16, name=f"kT{i}") for i in range(4)]
            v_sb = state.tile([P, NT, 8 * 65], BF16)  # [t_in, t_chunk, strip]

            # --- preamble: first x tiles ahead of everything, then weights
            xts = {}
            wvr = wv_d.rearrange("(eo p) c -> p eo c", p=P)
            wqr = wq_d.rearrange("(eo p) c -> p eo c", p=P)
            wkr = wk_d.rearrange("(eo p) c -> p eo c", p=P)
            for ti in range(2):
                xt = xin.tile([P, E], F32, tag="xt", name=f"xt{ti}")
                nc.sync.dma_start(xt[:], x_d[ti * P : (ti + 1) * P, :])
                xts[ti] = xt
            for g in range(2):
                sl = slice(g * 4, (g + 1) * 4)
                nc.sync.dma_start(wv_sb[:, sl, :], wvr[:, sl, :])
            for ti in range(2, 4):
                xt = xin.tile([P, E], F32, tag="xt", name=f"xt{ti}")
                nc.sync.dma_start(xt[:], x_d[ti * P : (ti + 1) * P, :])
                xts[ti] = xt
            nc.sync.dma_start(tri_sb[:], tri_d[:])
            make_identity(nc, ident)
            nc.vector.memset(eps_t[:], EPS)
            nc.vector.memset(
                v_sb[:].rearrange("p t (h c) -> p t h c", c=65)[:, :, :, 64:65], 1.0
            )
            for g in range(2):
                sl = slice(g * 4, (g + 1) * 4)
                nc.sync.dma_start(wq_sb[:, sl, :], wqr[:, sl, :])
                nc.sync.dma_start(wk_sb[:, sl, :], wkr[:, sl, :])
            nc.sync.dma_start(bq_sb[:], bq_d.rearrange("(cc p) -> p cc", p=P))
            nc.sync.dma_start(bk_sb[:], bk_d.rearrange("(cc p) -> p cc", p=P))
            wor = wo_d.rearrange("(co p) e -> p co e", p=P)
            for g in range(2):
                sl = slice(g * 2, (g + 1) * 2)
                nc.sync.dma_start(wo_sb[:, sl, :], wor[:, sl, :])

            # PE warm-up: dummy transposes ramp the tensor-engine
            # p-state while the first x tiles and LN are still in flight.
            psW = psA.tile([P, P], BF16, tag="a", name="psW")
            for _ in range(80):
                nc.tensor.transpose(psW[:], ident[:], ident)

            # per-tsl live tiles
            hTs = {}  # tsl -> [P, 8, 512]
            qts = {}  # tsl -> [P, 4, 512]
            pts = {}  # (tsl, hc) -> [P, NT, 2, 512]
            yTs = {}  # tsl -> [P, 4, 512]

            # ---------------- chunk emitters ----------------
            def A_ti(tsl, ti):
                """LN + transpose + V-projection for token tile ti."""
                loc = (ti - tsl * 4) * P
                if ti in xts:
                    xt = xts.pop(ti)
                else:
                    xt = xin.tile([P, E], F32, tag="xt")
                    nc.sync.dma_start(xt[:], x_d[ti * P : (ti + 1) * P, :])
                ht = hpool.tile([P, E], BF16)
                _ln_tile(nc, small, xt[:], ht[:], eps_t)
                hT = hTs[tsl]
                for g in range(2):
                    trp = psA.tile([P, 4, P], BF16, tag="a")
                    for j in range(4):
                        ec = g * 4 + j
                        nc.tensor.transpose(
                            trp[:, j, :], ht[:, ec * P : (ec + 1) * P], ident
                        )
                    nc.vector.tensor_copy(
                        hT[:, g * 4 : (g + 1) * 4, loc : loc + P], trp[:]
                    )
                psv = psA.tile([P, 512], F32, tag="a")
                for ec in range(8):
                    nc.tensor.matmul(
                        psv[:],
                        lhsT=hT[:, ec, loc : loc + P],
                        rhs=wv_sb[:, ec, :],
                        start=(ec == 0),
                        stop=(ec == 7),
                    )
                nc.vector.tensor_copy(
                    v_sb[:, ti, :].rearrange("p (h c) -> p h c", c=65)[:, :, 0:64],
                    psv[:].rearrange("p (h c) -> p h c", c=64),
                )

            def A_qk(tsl, cc):
                """Q and K projection for c-chunk cc of slice tsl."""
                hT = hTs[tsl]
                psq = psA.tile([P, 512], F32, tag="a")
                psk = psA.tile([P, 512], F32, tag="a")
                for ec in range(8):
                    nc.tensor.matmul(
                        psq[:],
                        lhsT=wq_sb[:, ec, cc * P : (cc + 1) * P],
                        rhs=hT[:, ec, :],
                        start=(ec == 0),
                        stop=(ec == 7),
                    )
                    nc.tensor.matmul(
                        psk[:],
                        lhsT=wk_sb[:, ec, cc * P : (cc + 1) * P],
                        rhs=hT[:, ec, :],
                        start=(ec == 0),
                        stop=(ec == 7),
                    )
                nc.vector.tensor_scalar(
                    out=qts[tsl][:, cc, :],
                    in0=psq[:],
                    scalar1=bq_sb[:, cc : cc + 1],
                    scalar2=None,
                    op0=OP.add,
                )
                nc.vector.tensor_scalar(
                    out=kTs[cc][:, tsl * 512 : (tsl + 1) * 512],
                    in0=psk[:],
                    scalar1=bk_sb[:, cc : cc + 1],
                    scalar2=None,
                    op0=OP.add,
                )

            def B_sc(tsl, hc, par, g):
                """Scores + exp for kb pair g, head parity par of pair hc.

                For diagonal kb blocks (kb >= 4*tsl) only q columns at or
                beyond the block's diagonal offset are ever consumed by
                B_av, so the matmul and exp are restricted accordingly.
                """
                kT = kTs[hc]
                psX = psS.tile([P, 2, 512], F32, tag="s")
                for j in range(2):
                    kb = g * 2 + j
                    off = max(0, kb - 4 * tsl) * P
                    ksl = slice(kb * P, (kb + 1) * P)
                    nc.tensor.matmul(
                        psX[:, j, off:512],
                        lhsT=kT[par * 64 : (par + 1) * 64, ksl],
                        rhs=qts[tsl][par * 64 : (par + 1) * 64, hc, off:512],
                        start=True,
                        stop=True,
                    )
                goff = max(0, g * 2 - 4 * tsl) * P
                nc.scalar.activation(
                    pts[(tsl, hc)][:, g * 2 : (g + 1) * 2, par, goff:512],
                    psX[:, :, goff:512],
                    AF.Exp,
                    scale=0.125,
                )

            def AVq(tsl, hc, qt):
                """AV accumulation for q tile qt; returns its PSUM tile."""
                loc = (qt - tsl * 4) * P
                pt = pts[(tsl, hc)]
                ptm = ptmpool.tile([P, 2, P], BF16, tag="ptm")
                nc.vector.tensor_tensor(
                    out=ptm[:],
                    in0=pt[:, qt, :, loc : loc + P],
                    in1=tri_sb[:],
                    op=OP.mult,
                )
                psy = psM.tile([P, 2, 65], F32, tag="m")
                for par in range(2):
                    strip = slice((2 * hc + par) * 65, (2 * hc + par) * 65 + 65)
                    for kb in range(qt + 1):
                        lhsT = (
                            ptm[:, par, :]
                            if kb == qt
                            else pt[:, kb, par, loc : loc + P]
                        )
                        nc.tensor.matmul(
                            psy[:, par, :],
                            lhsT=lhsT,
                            rhs=v_sb[:, kb, strip],
                            start=(kb == 0),
                            stop=(kb == qt),
                        )
                return psy

            def NORMq(tsl, hc, qt, psy):
                """Normalize + transpose q tile qt into yT (c-major).

                Emitted one q tile behind AVq so the ytr transpose's wait
                on the DVE normalize hides behind the next AV chain.
                """
                loc = (qt - tsl * 4) * P
                rec = sumspool.tile([P, 2, 1], F32, tag="rec")
                nc.vector.reciprocal(rec[:], psy[:, :, 64:65])
                yqt = yqpool.tile([P, P], BF16)
                for par in range(2):
                    nc.vector.tensor_scalar(
                        out=yqt[:, par * 64 : (par + 1) * 64],
                        in0=psy[:, par, 0:64],
                        scalar1=rec[:, par, :],
                        scalar2=None,
                        op0=OP.mult,
                    )
                psT = psA.tile([P, P], BF16, tag="a")
                nc.tensor.transpose(psT[:], yqt[:], ident)
                nc.vector.tensor_copy(yTs[tsl][:, hc, loc : loc + P], psT[:])

            def B_av(tsl, hc):
                """AV + normalize for all 4 q tiles of head pair hc."""
                pend = []
                for q in range(tsl * 4, tsl * 4 + 4):
                    psy = AVq(tsl, hc, q)
                    pend.append((q, psy))
                    if len(pend) > 1:
                        q0, psy0 = pend.pop(0)
                        NORMq(tsl, hc, q0, psy0)
                while pend:
                    q0, psy0 = pend.pop(0)
                    NORMq(tsl, hc, q0, psy0)

            def O_ti(tsl, ti, eg):
                """O-projection for token tile ti, embed half eg."""
                loc = (ti - tsl * 4) * P
                yT = yTs[tsl]
                pso = psA.tile([P, 512], F32, tag="a")
                for cc in range(4):
                    nc.tensor.matmul(
                        pso[:],
                        lhsT=yT[:, cc, loc : loc + P],
                        rhs=wo_sb[:, cc, eg * 512 : (eg + 1) * 512],
                        start=(cc == 0),
                        stop=(cc == 3),
                    )
                ot = yqpool.tile([P, 512], BF16, tag="ot")
                nc.vector.tensor_copy(ot[:], pso[:])
                nc.sync.dma_start(
                    out_d[ti * P : (ti + 1) * P, eg * 512 : (eg + 1) * 512], ot[:]
                )

            # ---------------- schedule ----------------
            def prep_slice(tsl):
                hTs[tsl] = hTpool.tile([P, 8, 512], BF16, tag="hT", name=f"hT{tsl}")
                qts[tsl] = qTpool.tile([P, 4, 512], BF16, tag="qT", name=f"qT{tsl}")
                yTs[tsl] = yTpool.tile([P, 4, 512], BF16, tag="yT", name=f"yT{tsl}")

            prep_slice(0)
            for ti in range(4):
                A_ti(0, ti)
            for cc in range(4):
                A_qk(0, cc)

            for tsl in range(NQ):
                nkb = 4 * tsl + 4
                # filler chunks emitted between score groups so the PE
                # never parks while ACT drains exps: next slice's LN/QKV
                # in regions 0-2; ALL deferred O-projections in region 3
                # (the most ACT-bound region, where exp load peaks).
                fillers = []
                if tsl + 1 < NQ:
                    prep_slice(tsl + 1)
                    for ti in range((tsl + 1) * 4, (tsl + 1) * 4 + 4):
                        xt = xin.tile([P, E], F32, tag="xt", name=f"xt{ti}")
                        nc.sync.dma_start(xt[:], x_d[ti * P : (ti + 1) * P, :])
                        xts[ti] = xt
                        fillers.append(("A", tsl + 1, ti, None))
                    for cc in range(4):
                        fillers.append(("Q", tsl + 1, cc, None))
                else:
                    for ts0 in range(3):
                        for ti in range(ts0 * 4, ts0 * 4 + 4):
                            for eg in range(2):
                                fillers.append(("O", ts0, ti, eg))

                def emit_filler():
                    if not fillers:
                        return
                    kind, a, b, c = fillers.pop(0)
                    if kind == "O":
                        O_ti(a, b, c)
                    elif kind == "A":
                        A_ti(a, b)
                    else:
                        A_qk(a, b)

                # score groups per (hc, par): nkb//2 of them; spread the
                # fillers evenly across them (Bresenham) so the PE always
                # has non-score work between groups while ACT drains exps.
                ngroups = 4 * 2 * (nkb // 2)
                nfill = len(fillers)
                gcount = 0
                npop = 0
                for hc in range(4):
                    pts[(tsl, hc)] = ptpool.tile(
                        [P, nkb, 2, 512], BF16, tag="pt", name=f"pt{tsl}_{hc}"
                    )
                    for par in range(2):
                        for g in range(nkb // 2):
                            B_sc(tsl, hc, par, g)
                            gcount += 1
                            while npop * ngroups < gcount * nfill:
                                emit_filler()
                                npop += 1
                    if hc >= 1:
                        B_av(tsl, hc - 1)
                pend3 = []
                for qt in range(tsl * 4, tsl * 4 + 4):
                    psy = AVq(tsl, 3, qt)
                    pend3.append((qt, psy))
                    if len(pend3) > 1:
                        q0, psy0 = pend3.pop(0)
                        NORMq(tsl, 3, q0, psy0)
                        if tsl == NQ - 1:
                            # yT column q0 is complete: project it out now
                            for eg in range(2):
                                O_ti(tsl, q0, eg)
                    emit_filler()
                while pend3:
                    q0, psy0 = pend3.pop(0)
                    NORMq(tsl, 3, q0, psy0)
                    if tsl == NQ - 1:
                        for eg in range(2):
                            O_ti(tsl, q0, eg)
                while fillers:
                    emit_filler()
    nc.compile()
    return nc


def build_ffn():
    """Launch 2: LN2 + GELU MLP + residual on a 1024-token slice.

    inputs : x2[1024,E] f32, w1[E,FF] bf16, w2[FF,E] bf16, b1[FF] f32
    output : out[1024,E] f32  (= x2 + gelu(LN(x2) @ w1 + b1) @ w2)

    ff1 is computed transposed (gT[f,t]) so the gelu output feeds the second
    matmul as lhsT without a transpose. W1 is loaded once; each chunk serves
    both 512-token halves. W2 arrives in chunks interleaved late (it is not
    needed until phase C), so startup DMA bandwidth goes to x and W1.
    """
    T = 1024
    nc = bacc.Bacc("TRN2", target_bir_lowering=False, debug=False, num_devices=8)
    x2_d = nc.dram_tensor("x2", [T, E], F32, kind="ExternalInput")
    w1_d = nc.dram_tensor("w1", [E, FF], BF16, kind="ExternalInput")
    w2_d = nc.dram_tensor("w2", [FF, E], BF16, kind="ExternalInput")
    b1_d = nc.dram_tensor("b1", [FF], F32, kind="ExternalInput")
    out_d = nc.dram_tensor("out", [T, E], F32, kind="ExternalOutput")

    NT = T // P  # 8 token tiles
    NF = FF // P  # 32 f chunks
    NFG = FF // 256  # 16 w1 dma chunks

    with tile.TileContext(nc) as tc:
        with (
            tc.tile_pool(name="consts", bufs=1) as consts,
            tc.tile_pool(name="state", bufs=1) as state,
            tc.tile_pool(name="w1p", bufs=6) as w1pool,
            tc.tile_pool(name="xin", bufs=4) as xin,
            tc.tile_pool(name="hp", bufs=2) as hpool,
            tc.tile_pool(name="outp", bufs=3) as outp,
            tc.tile_pool(name="small", bufs=6) as small,
            tc.tile_pool(name="psB", bufs=2, space="PSUM") as psB,
            tc.tile_pool(name="psC", bufs=4, space="PSUM") as psC,
        ):
            ident = consts.tile([P, P], BF16)
            eps_t = consts.tile([P, 1], F32)
            w2_sb = consts.tile([P, NF, E], BF16)
            b1_sb = consts.tile([P, NF], F32)
            h2T = state.tile([P, 8, T], BF16)  # [e_in, e_chunk, t]
            gT = state.tile([P, NF, T], BF16)  # [f_in, f_chunk, t]

            # preamble: x tiles first, then identity/eps/b1; w2 is emitted
            # in chunks interleaved into the ff1 loop below.
            xts = {}

            def issue_x(ti):
                xt = xin.tile([P, E], F32, tag="xt", name=f"xt{ti}")
                nc.sync.dma_start(xt[:], x2_d[ti * P : (ti + 1) * P, :])
                xts[ti] = xt

            for ti in range(4):
                issue_x(ti)
            make_identity(nc, ident)
            nc.vector.memset(eps_t[:], EPS)
            nc.sync.dma_start(b1_sb[:], b1_d.rearrange("(fo p) -> p fo", p=P))
            w1r = w1_d.rearrange("(eo p) f -> p eo f", p=P)
            w2r = w2_d.rearrange("(fo p) e -> p fo e", p=P)

            w1g_tiles = {}

            def issue_w1(fg):
                t = w1pool.tile([P, 8, 256], BF16, tag="w1")
                nc.sync.dma_start(t[:], w1r[:, :, fg * 256 : (fg + 1) * 256])
                w1g_tiles[fg] = t

            issue_w1(0)
            issue_w1(1)

            # PE warm-up: ramp the tensor-engine p-state while the first
            # x tiles and LN are still in flight.
            psW = psB.tile([P, P], BF16, tag="u", name="psW")
            for _ in range(60):
                nc.tensor.transpose(psW[:], ident[:], ident)

            def A_ti(to):
                xt = xts.pop(to)
                h2 = hpool.tile([P, E], BF16)
                _ln_tile(nc, small, xt[:], h2[:], eps_t)
                for g in range(2):
                    trp = psB.tile([P, 4, P], BF16, tag="u")
                    for j in range(4):
                        ec = g * 4 + j
                        nc.tensor.transpose(
                            trp[:, j, :], h2[:, ec * P : (ec + 1) * P], ident
                        )
                    nc.vector.tensor_copy(
                        h2T[:, g * 4 : (g + 1) * 4, to * P : (to + 1) * P],
                        trp[:],
                    )

            def ff1(fg, tsl, w1g):
                tofs = tsl * 512
                ps0 = psB.tile([P, 2, 512], F32, tag="u")
                for ec in range(8):
                    for j in range(2):
                        nc.tensor.matmul(
                            ps0[:, j, :],
                            lhsT=w1g[:, ec, j * P : (j + 1) * P],
                            rhs=h2T[:, ec, tofs : tofs + 512],
                            start=(ec == 0),
                            stop=(ec == 7),
                        )
                for j in range(2):
                    fc = fg * 2 + j
                    nc.scalar.activation(
                        gT[:, fc, tofs : tofs + 512],
                        ps0[:, j, :],
                        AF.Gelu,
                        bias=b1_sb[:, fc : fc + 1],
                    )

            def issue_w2(fg):
                if fg % 2 == 0 and fg // 2 < 8:
                    wsl = slice((fg // 2) * 4, (fg // 2) * 4 + 4)
                    nc.sync.dma_start(w2_sb[:, wsl, :], w2r[:, wsl, :])

            # ---- Phase A/B interleaved: LN+transpose first 4 tiles, then
            # ff1 on the first half while the second half's LN runs; the
            # first 4 W1 chunks stay resident so their tsl=1 pass follows.
            for to in range(4):
                A_ti(to)
            held = {}
            for fg in range(4):
                w1g = w1g_tiles.pop(fg)
                if fg + 2 < NFG:
                    issue_w1(fg + 2)
                issue_w2(fg)
                issue_x(4 + fg)
                ff1(fg, 0, w1g)
                A_ti(4 + fg)
                held[fg] = w1g
            for fg in range(4):
                ff1(fg, 1, held.pop(fg))
            for fg in range(4, NFG):
                w1g = w1g_tiles.pop(fg)
                if fg + 2 < NFG:
                    issue_w1(fg + 2)
                issue_w2(fg)
                ff1(fg, 0, w1g)
                ff1(fg, 1, w1g)

            # ---- Phase C: out = x2 + gT^T @ W2 ----
            for tb in range(NT):
                psO = psC.tile([P, 512], F32, tag="c")
                psP = psC.tile([P, 512], F32, tag="c")
                for fc in range(NF):
                    nc.tensor.matmul(
                        psO[:],
                        lhsT=gT[:, fc, tb * P : (tb + 1) * P],
                        rhs=w2_sb[:, fc, 0:512],
                        start=(fc == 0),
                        stop=(fc == NF - 1),
                    )
                    nc.tensor.matmul(
                        psP[:],
                        lhsT=gT[:, fc, tb * P : (tb + 1) * P],
                        rhs=w2_sb[:, fc, 512:1024],
                        start=(fc == 0),
                        stop=(fc == NF - 1),
                    )
                xr = xin.tile([P, E], F32, tag="xt")
                nc.sync.dma_start(xr[:], x2_d[tb * P : (tb + 1) * P, :])
                for eg, psX in ((0, psO), (1, psP)):
                    ot = outp.tile([P, 512], F32)
                    nc.vector.tensor_tensor(
                        out=ot[:],
                        in0=psX[:],
                        in1=xr[:, eg * 512 : (eg + 1) * 512],
                        op=OP.add,
                    )
                    nc.sync.dma_start(
                        out_d[tb * P : (tb + 1) * P, eg * 512 : (eg + 1) * 512],
                        ot[:],
                    )
    nc.compile()
    return nc


# ---------------------------------------------------------------------------
# Host orchestration
# ---------------------------------------------------------------------------


def _bf16(a):
    return np.ascontiguousarray(np.asarray(a, dtype=np.float32)).astype(BF16NP)


def _f32(a):
    return np.ascontiguousarray(np.asarray(a, dtype=np.float32))


def _tri01():
    kp = np.arange(P)[:, None]
    qf = np.arange(P)[None, :]
    t = (kp <= qf).astype(np.float32)
    return np.ascontiguousarray(np.stack([t, t], axis=1)).astype(BF16NP)


def kernel(
    x, Wq, bq, Wk, bk, Wv, bv, Wo, bo, g1, beta1, g2, beta2, W1, b1, W2, b2
):
    out, _ = _run(
        x, Wq, bq, Wk, bk, Wv, bv, Wo, bo, g1, beta1, g2, beta2, W1, b1, W2, b2
    )
    return out


def _run(
    x, Wq, bq, Wk, bk, Wv, bv, Wo, bo, g1, beta1, g2, beta2, W1, b1, W2, b2,
    trace=False,
):
    x = _f32(x)
    Wq, bq = _f32(Wq), _f32(bq)
    Wk, bk = _f32(Wk), _f32(bk)
    Wv, bv = _f32(Wv), _f32(bv)
    Wo, bo = _f32(Wo), _f32(bo)
    g1, beta1 = _f32(g1), _f32(beta1)
    g2, beta2 = _f32(g2), _f32(beta2)
    W1, b1 = _f32(W1), _f32(b1)
    W2, b2 = _f32(W2), _f32(b2)

    # Fold LN1 affine into the QKV projections: h = ln0*g1+beta1 =>
    # h@W + b == ln0@(g1[:,None]*W) + (beta1@W + b)
    Wq_e, bq_e = Wq * g1[:, None], beta1 @ Wq + bq
    Wk_e, bk_e = Wk * g1[:, None], beta1 @ Wk + bk
    Wv_e, bv_e = Wv * g1[:, None], beta1 @ Wv + bv
    # V-bias rides through the attention average (rows of attn sum to 1):
    # y = P@(v + bv) = P@v + bv  =>  fold bv@Wo into the residual bias.
    bo_e = bo + bv_e @ Wo
    # Fold LN2 affine into W1.
    W1_e, b1_e = W1 * g2[:, None], beta2 @ W1 + b1

    tri = _tri01()
    nc1 = build_attn()
    in_maps1 = []
    for c in range(8):
        b_, hh = c // 2, c % 2
        cs = 512 * hh
        in_maps1.append(
            {
                "x": x[b_],
                "wq": _bf16(Wq_e[:, cs : cs + 512]),
                "wk": _bf16(Wk_e[:, cs : cs + 512]),
                "wv": _bf16(Wv_e[:, cs : cs + 512]),
                "wo": _bf16(Wo[cs : cs + 512, :]),
                "bq": bq_e[cs : cs + 512],
                "bk": bk_e[cs : cs + 512],
                "tri": tri,
            }
        )
    res1 = run_bass_kernel_spmd(nc1, in_maps1, list(range(8)), trace=trace)
    x2 = x + bo_e[None, None, :]
    for c in range(8):
        x2[c // 2] += np.asarray(res1.results[c]["out"], dtype=np.float32)

    x2f = np.ascontiguousarray(x2.reshape(B * S, E), dtype=np.float32)
    w1b, w2b = _bf16(W1_e), _bf16(W2)
    nc2 = build_ffn()
    in_maps2 = [
        {
            "x2": x2f[c * 1024 : (c + 1) * 1024],
            "w1": w1b,
            "w2": w2b,
            "b1": b1_e,
        }
        for c in range(8)
    ]
    res2 = run_bass_kernel_spmd(nc2, in_maps2, list(range(8)), trace=trace)
    out = np.concatenate([res2.results[c]["out"] for c in range(8)], axis=0)
    out = out + b2[None, :]
    times = (res1.exec_time_ns, res2.exec_time_ns)
    return out.reshape(B, S, E).astype(np.float32), times


# revision 46
# speedup vs baseline: 1.0043x; 1.0043x over previous
"""Trainium2 Bass kernel for a pre-LN causal decoder block.

Model: B=4, S=2048, EMBED=1024, HEADS=16, HEAD_DIM=64, FF=4096, fp32 I/O.

Sharding (8 NeuronCores, two SPMD launches):
  Launch 1 (attention): core c -> batch b=c//2, head-half hh=c%2 (8 heads).
    Each core computes LN1, its 512-wide QKV column slice, causal attention
    for its 8 heads, and a partial O-projection. Host sums the two partials
    per batch and adds the residual + folded biases.
  Launch 2 (FFN): tokens (B*S=8192) sharded 8 ways (1024 tokens/core);
    each core runs LN2 + GELU MLP on its tokens with full (folded) W1/W2.

Optimization notes (vs the first working version; ~1013us -> ~556us):
  - AV matmul reoriented (lhsT=probability block, rhs=V strip + ones
    column) so the softmax denominators land as a PSUM *column*;
    normalization is a local per-partition reciprocal+multiply (the old
    DRAM broadcast round-trip is gone).
  - Causal mask matmuls removed entirely: AV consumes only kb <= qt
    blocks, so off-diagonal garbage is never read; the diagonal 128x128
    triangle blocks get a tiny DVE binary-mask multiply instead, and the
    diagonal score matmuls/exps are column-restricted to what is read.
  - Emission is software-pipelined: the next slice's LN/QKV fillers are
    Bresenham-spread between score/exp groups (regions 0-2) and ALL
    O-projections are deferred to region 3 (where exp load peaks), so
    the in-order PE queue never parks while the ACT engine drains exps.
    Normalization trails its AV chain by one q tile so its DVE latency
    hides behind the next chain. PE warm-up transposes ramp the clock
    during the initial x-DMA/LN window.
  - FFN: W1 loaded once (the first 4 chunks stay resident to serve both
    512-token halves while the second half's LN runs), startup DMAs
    reordered (x first, W2 split into late-interleaved chunks).

All matmuls run in bf16 with fp32 PSUM accumulation; LN statistics and
softmax run in fp32. LN affine params and all biases are folded into the
weight matrices / per-channel biases on the host, so the device kernels
implement the fully general module.
"""

import numpy as np
import ml_dtypes

# ---------------------------------------------------------------------------
# Environment patches (in-process only).
# ---------------------------------------------------------------------------


def _install_env():
    import sys
    import types

    # antenv.axon_hooks may be missing in this image; provide it so
    # run_bass_kernel_spmd(trace=True) degrades gracefully / can profile.
    try:
        import antenv.axon_hooks  # noqa: F401
    except ImportError:
        mod = types.ModuleType("antenv.axon_hooks")
        mod._hook = None
        mod.set_axon_ntff_profile_hook = lambda h: setattr(mod, "_hook", h)
        mod.get_axon_ntff_profile_hook = lambda: mod._hook
        sys.modules["antenv.axon_hooks"] = mod
        try:
            import antenv

            antenv.axon_hooks = mod
        except ImportError:
            pass

    import concourse.bass_utils as bu

    # zero-egress sandbox: don't try to copy NEFF dirs to a remote bucket
    bu.upload_artifacts = lambda tmpdir: tmpdir

    # This image's walrus accepts at most ONE sync-wait on a TPB_CTRL
    # (Drain/Nop) instruction; Tile's kernel-tail drain piles every
    # outstanding sem wait onto a single Drain and codegen fails with
    # "Too many sync wait commands". Split the waits across chained
    # single-wait nops (identical semantics: all waits complete on SP
    # before the all-engine barrier / semaphore reset).
    import concourse.mybir as mybir
    import concourse.tile as tile
    from concourse.vector_clock import ScopedClock

    if getattr(tile.TileContext, "_drain_patch_installed", False):
        return

    def _drain_and_barrier(self, tick_clock, wait_clock):
        nc = self.nc
        drain_inst = nc.sync.drain()
        wait_clock.add_sem_waits(
            drain_inst.ins, ScopedClock({None: tick_clock.global_clock})
        )
        si = drain_inst.ins.sync_info
        waits = list(si.on_wait or [])
        if len(waits) > 1:
            si.on_wait = waits[:1]
            for w in waits[1:]:
                nop = nc.sync.nop()
                nop.ins.sync_info = mybir.SyncInfo(on_wait=[w], on_update=[])
        nc.all_engine_barrier()
        assert self.sems is not None
        popped = nc._tile_sem_poison_stack.pop()
        assert popped is self._sem_poison
        nc.clear_and_free_semaphores(list(self.sems.allocated().values()))
        nc.all_engine_barrier()

    tile.TileContext._drain_and_barrier = _drain_and_barrier
    tile.TileContext._drain_patch_installed = True


_install_env()

import concourse.bass as bass  # noqa: E402
from concourse import bacc  # noqa: E402
import concourse.mybir as mybir  # noqa: E402
import concourse.tile as tile  # noqa: E402
from concourse.bass_utils import run_bass_kernel_spmd  # noqa: E402
from concourse.masks import make_identity  # noqa: E402

F32 = mybir.dt.float32
BF16 = mybir.dt.bfloat16
AF = mybir.ActivationFunctionType
OP = mybir.AluOpType
BF16NP = ml_dtypes.bfloat16

B, S, E, H, HD, FF = 4, 2048, 1024, 16, 64, 4096
P = 128
EPS = 1e-5


def _ln_tile(nc, pool_small, x_ap, out_ap, eps_tile):
    """Non-affine LayerNorm of one [128, E] tile; out may be bf16."""
    nsub = E // 512
    stats = pool_small.tile([P, nsub, 6], F32, tag="lnstats")
    for j in range(nsub):
        nc.vector.bn_stats(stats[:, j, :], x_ap[:, j * 512 : (j + 1) * 512])
    mv = pool_small.tile([P, 2], F32, tag="lnmv")
    nc.vector.bn_aggr(mv[:], stats[:])
    rstd = pool_small.tile([P, 1], F32, tag="lnrstd")
    nc.scalar.activation(rstd[:], mv[:, 1:2], AF.Sqrt, bias=eps_tile[:])
    nc.vector.reciprocal(rstd[:], rstd[:])
    nc.vector.tensor_scalar(
        out=out_ap,
        in0=x_ap,
        scalar1=mv[:, 0:1],
        scalar2=rstd[:],
        op0=OP.subtract,
        op1=OP.mult,
    )


def build_attn():
    """Launch 1: per-core attention partial.

    inputs : x[S,E] f32, wq/wk/wv[E,512] bf16, wo[512,E] bf16,
             bq/bk[512] f32, tri[P,2,P] bf16 (lower-triangular 0/1 x2)
    output : out[S,E] bf16  (= y_heads @ wo, partial over head-half;
             the host accumulates the two partials in f32)
    """
    nc = bacc.Bacc("TRN2", target_bir_lowering=False, debug=False, num_devices=8)
    x_d = nc.dram_tensor("x", [S, E], F32, kind="ExternalInput")
    wq_d = nc.dram_tensor("wq", [E, 512], BF16, kind="ExternalInput")
    wk_d = nc.dram_tensor("wk", [E, 512], BF16, kind="ExternalInput")
    wv_d = nc.dram_tensor("wv", [E, 512], BF16, kind="ExternalInput")
    wo_d = nc.dram_tensor("wo", [512, E], BF16, kind="ExternalInput")
    bq_d = nc.dram_tensor("bq", [512], F32, kind="ExternalInput")
    bk_d = nc.dram_tensor("bk", [512], F32, kind="ExternalInput")
    tri_d = nc.dram_tensor("tri", [P, 2, P], BF16, kind="ExternalInput")
    out_d = nc.dram_tensor("out", [S, E], BF16, kind="ExternalOutput")

    NT = S // P  # 16 token tiles
    NQ = S // 512  # 4 q slices

    with tile.TileContext(nc) as tc:
        with (
            tc.tile_pool(name="consts", bufs=1) as consts,
            tc.tile_pool(name="state", bufs=1) as state,
            tc.tile_pool(name="xin", bufs=4) as xin,
            tc.tile_pool(name="hp", bufs=2) as hpool,
            tc.tile_pool(name="hT", bufs=2) as hTpool,
            tc.tile_pool(name="qT", bufs=2) as qTpool,
            tc.tile_pool(name="pt", bufs=2) as ptpool,
            tc.tile_pool(name="ptm", bufs=3) as ptmpool,
            tc.tile_pool(name="yq", bufs=3) as yqpool,
            tc.tile_pool(name="yT", bufs=4) as yTpool,
            tc.tile_pool(name="sums", bufs=4) as sumspool,
            tc.tile_pool(name="small", bufs=6) as small,
            tc.tile_pool(name="psS", bufs=2, space="PSUM") as psS,
            tc.tile_pool(name="psA", bufs=2, space="PSUM") as psA,
            tc.tile_pool(name="psM", bufs=2, space="PSUM") as psM,
        ):
            ident = consts.tile([P, P], BF16)
            eps_t = consts.tile([P, 1], F32)
            tri_sb = consts.tile([P, 2, P], BF16)
            wq_sb = consts.tile([P, 8, 512], BF16)
            wk_sb = consts.tile([P, 8, 512], BF16)
            wv_sb = consts.tile([P, 8, 512], BF16)
            wo_sb = consts.tile([P, 4, E], BF16)
            bq_sb = consts.tile([P, 4], F32)
            bk_sb = consts.tile([P, 4], F32)

            kTs = [state.tile([P, S], BF16, name=f"kT{i}") for i in range(4)]
            v_sb = state.tile([P, NT, 8 * 65], BF16)  # [t_in, t_chunk, strip]

            # --- preamble: first x tiles ahead of everything, then weights
            xts = {}
            wvr = wv_d.rearrange("(eo p) c -> p eo c", p=P)
            wqr = wq_d.rearrange("(eo p) c -> p eo c", p=P)
            wkr = wk_d.rearrange("(eo p) c -> p eo c", p=P)
            for ti in range(2):
                xt = xin.tile([P, E], F32, tag="xt", name=f"xt{ti}")
                nc.sync.dma_start(xt[:], x_d[ti * P : (ti + 1) * P, :])
                xts[ti] = xt
            for g in range(2):
                sl = slice(g * 4, (g + 1) * 4)
                nc.sync.dma_start(wv_sb[:, sl, :], wvr[:, sl, :])
            for ti in range(2, 4):
                xt = xin.tile([P, E], F32, tag="xt", name=f"xt{ti}")
                nc.sync.dma_start(xt[:], x_d[ti * P : (ti + 1) * P, :])
                xts[ti] = xt
            nc.sync.dma_start(tri_sb[:], tri_d[:])
            make_identity(nc, ident)
            nc.vector.memset(eps_t[:], EPS)
            nc.vector.memset(
                v_sb[:].rearrange("p t (h c) -> p t h c", c=65)[:, :, :, 64:65], 1.0
            )
            for g in range(2):
                sl = slice(g * 4, (g + 1) * 4)
                nc.sync.dma_start(wq_sb[:, sl, :], wqr[:, sl, :])
                nc.sync.dma_start(wk_sb[:, sl, :], wkr[:, sl, :])
            nc.sync.dma_start(bq_sb[:], bq_d.rearrange("(cc p) -> p cc", p=P))
            nc.sync.dma_start(bk_sb[:], bk_d.rearrange("(cc p) -> p cc", p=P))
            wor = wo_d.rearrange("(co p) e -> p co e", p=P)
            for g in range(2):
                sl = slice(g * 2, (g + 1) * 2)
                nc.sync.dma_start(wo_sb[:, sl, :], wor[:, sl, :])

            # PE warm-up: dummy transposes ramp the tensor-engine
            # p-state while the first x tiles and LN are still in flight.
            psW = psA.tile([P, P], BF16, tag="a", name="psW")
            for _ in range(80):
                nc.tensor.transpose(psW[:], ident[:], ident)

            # per-tsl live tiles
            hTs = {}  # tsl -> [P, 8, 512]
            qts = {}  # tsl -> [P, 4, 512]
            pts = {}  # (tsl, hc) -> [P, NT, 2, 512]
            yTs = {}  # tsl -> [P, 4, 512]

            # ---------------- chunk emitters ----------------
            def A_ti(tsl, ti):
                """LN + transpose + V-projection for token tile ti."""
                loc = (ti - tsl * 4) * P
                if ti in xts:
                    xt = xts.pop(ti)
                else:
                    xt = xin.tile([P, E], F32, tag="xt")
                    nc.sync.dma_start(xt[:], x_d[ti * P : (ti + 1) * P, :])
                ht = hpool.tile([P, E], BF16)
                _ln_tile(nc, small, xt[:], ht[:], eps_t)
                hT = hTs[tsl]
                for g in range(2):
                    trp = psA.tile([P, 4, P], BF16, tag="a")
                    for j in range(4):
                        ec = g * 4 + j
                        nc.tensor.transpose(
                            trp[:, j, :], ht[:, ec * P : (ec + 1) * P], ident
                        )
                    nc.vector.tensor_copy(
                        hT[:, g * 4 : (g + 1) * 4, loc : loc + P], trp[:]
                    )
                psv = psA.tile([P, 512], F32, tag="a")
                for ec in range(8):
                    nc.tensor.matmul(
                        psv[:],
                        lhsT=hT[:, ec, loc : loc + P],
                        rhs=wv_sb[:, ec, :],
                        start=(ec == 0),
                        stop=(ec == 7),
                    )
                nc.vector.tensor_copy(
                    v_sb[:, ti, :].rearrange("p (h c) -> p h c", c=65)[:, :, 0:64],
                    psv[:].rearrange("p (h c) -> p h c", c=64),
                )

            def A_qk(tsl, cc):
                """Q and K projection for c-chunk cc of slice tsl."""
                hT = hTs[tsl]
                psq = psA.tile([P, 512], F32, tag="a")
                psk = psA.tile([P, 512], F32, tag="a")
                for ec in range(8):
                    nc.tensor.matmul(
                        psq[:],
                        lhsT=wq_sb[:, ec, cc * P : (cc + 1) * P],
                        rhs=hT[:, ec, :],
                        start=(ec == 0),
                        stop=(ec == 7),
                    )
                    nc.tensor.matmul(
                        psk[:],
                        lhsT=wk_sb[:, ec, cc * P : (cc + 1) * P],
                        rhs=hT[:, ec, :],
                        start=(ec == 0),
                        stop=(ec == 7),
                    )
                nc.vector.tensor_scalar(
                    out=qts[tsl][:, cc, :],
                    in0=psq[:],
                    scalar1=bq_sb[:, cc : cc + 1],
                    scalar2=None,
                    op0=OP.add,
                )
                nc.vector.tensor_scalar(
                    out=kTs[cc][:, tsl * 512 : (tsl + 1) * 512],
                    in0=psk[:],
                    scalar1=bk_sb[:, cc : cc + 1],
                    scalar2=None,
                    op0=OP.add,
                )

            def B_sc(tsl, hc, par, g):
                """Scores + exp for kb pair g, head parity par of pair hc.

                For diagonal kb blocks (kb >= 4*tsl) only q columns at or
                beyond the block's diagonal offset are ever consumed by
                B_av, so the matmul and exp are restricted accordingly.
                """
                kT = kTs[hc]
                psX = psS.tile([P, 2, 512], F32, tag="s")
                for j in range(2):
                    kb = g * 2 + j
                    off = max(0, kb - 4 * tsl) * P
                    ksl = slice(kb * P, (kb + 1) * P)
                    nc.tensor.matmul(
                        psX[:, j, off:512],
                        lhsT=kT[par * 64 : (par + 1) * 64, ksl],
                        rhs=qts[tsl][par * 64 : (par + 1) * 64, hc, off:512],
                        start=True,
                        stop=True,
                    )
                goff = max(0, g * 2 - 4 * tsl) * P
                nc.scalar.activation(
                    pts[(tsl, hc)][:, g * 2 : (g + 1) * 2, par, goff:512],
                    psX[:, :, goff:512],
                    AF.Exp,
                    scale=0.125,
                )

            def AVq(tsl, hc, qt):
                """AV accumulation for q tile qt; returns its PSUM tile."""
                loc = (qt - tsl * 4) * P
                pt = pts[(tsl, hc)]
                ptm = ptmpool.tile([P, 2, P], BF16, tag="ptm")
                nc.vector.tensor_tensor(
                    out=ptm[:],
                    in0=pt[:, qt, :, loc : loc + P],
                    in1=tri_sb[:],
                    op=OP.mult,
                )
                psy = psM.tile([P, 2, 65], F32, tag="m")
                for par in range(2):
                    strip = slice((2 * hc + par) * 65, (2 * hc + par) * 65 + 65)
                    for kb in range(qt + 1):
                        lhsT = (
                            ptm[:, par, :]
                            if kb == qt
                            else pt[:, kb, par, loc : loc + P]
                        )
                        nc.tensor.matmul(
                            psy[:, par, :],
                            lhsT=lhsT,
                            rhs=v_sb[:, kb, strip],
                            start=(kb == 0),
                            stop=(kb == qt),
                        )
                return psy

            def NORMq(tsl, hc, qt, psy):
                """Normalize + transpose q tile qt into yT (c-major).

                Emitted one q tile behind AVq so the ytr transpose's wait
                on the DVE normalize hides behind the next AV chain.
                """
                loc = (qt - tsl * 4) * P
                rec = sumspool.tile([P, 2, 1], F32, tag="rec")
                nc.vector.reciprocal(rec[:], psy[:, :, 64:65])
                yqt = yqpool.tile([P, P], BF16)
                for par in range(2):
                    nc.vector.tensor_scalar(
                        out=yqt[:, par * 64 : (par + 1) * 64],
                        in0=psy[:, par, 0:64],
                        scalar1=rec[:, par, :],
                        scalar2=None,
                        op0=OP.mult,
                    )
                psT = psA.tile([P, P], BF16, tag="a")
                nc.tensor.transpose(psT[:], yqt[:], ident)
                nc.vector.tensor_copy(yTs[tsl][:, hc, loc : loc + P], psT[:])

            def B_av(tsl, hc):
                """AV + normalize for all 4 q tiles of head pair hc."""
                pend = []
                for q in range(tsl * 4, tsl * 4 + 4):
                    psy = AVq(tsl, hc, q)
                    pend.append((q, psy))
                    if len(pend) > 1:
                        q0, psy0 = pend.pop(0)
                        NORMq(tsl, hc, q0, psy0)
                while pend:
                    q0, psy0 = pend.pop(0)
                    NORMq(tsl, hc, q0, psy0)

            def O_ti(tsl, ti, eg):
                """O-projection for token tile ti, embed half eg."""
                loc = (ti - tsl * 4) * P
                yT = yTs[tsl]
                pso = psA.tile([P, 512], F32, tag="a")
                for cc in range(4):
                    nc.tensor.matmul(
                        pso[:],
                        lhsT=yT[:, cc, loc : loc + P],
                        rhs=wo_sb[:, cc, eg * 512 : (eg + 1) * 512],
                        start=(cc == 0),
                        stop=(cc == 3),
                    )
                ot = yqpool.tile([P, 512], BF16, tag="ot")
                nc.vector.tensor_copy(ot[:], pso[:])
                nc.sync.dma_start(
                    out_d[ti * P : (ti + 1) * P, eg * 512 : (eg + 1) * 512], ot[:]
                )

            # ---------------- schedule ----------------
            def prep_slice(tsl):
                hTs[tsl] = hTpool.tile([P, 8, 512], BF16, tag="hT", name=f"hT{tsl}")
                qts[tsl] = qTpool.tile([P, 4, 512], BF16, tag="qT", name=f"qT{tsl}")
                yTs[tsl] = yTpool.tile([P, 4, 512], BF16, tag="yT", name=f"yT{tsl}")

            prep_slice(0)
            for ti in range(4):
                A_ti(0, ti)
            for cc in range(4):
                A_qk(0, cc)

            for tsl in range(NQ):
                nkb = 4 * tsl + 4
                # filler chunks emitted between score groups so the PE
                # never parks while ACT drains exps: next slice's LN/QKV
                # in regions 0-2; ALL deferred O-projections in region 3
                # (the most ACT-bound region, where exp load peaks).
                fillers = []
                if tsl + 1 < NQ:
                    prep_slice(tsl + 1)
                    for ti in range((tsl + 1) * 4, (tsl + 1) * 4 + 4):
                        xt = xin.tile([P, E], F32, tag="xt", name=f"xt{ti}")
                        nc.sync.dma_start(xt[:], x_d[ti * P : (ti + 1) * P, :])
                        xts[ti] = xt
                        fillers.append(("A", tsl + 1, ti, None))
                    for cc in range(4):
                        fillers.append(("Q", tsl + 1, cc, None))
                else:
                    for ts0 in range(3):
                        for ti in range(ts0 * 4, ts0 * 4 + 4):
                            for eg in range(2):
                                fillers.append(("O", ts0, ti, eg))

                def emit_filler():
                    if not fillers:
                        return
                    kind, a, b, c = fillers.pop(0)
                    if kind == "O":
                        O_ti(a, b, c)
                    elif kind == "A":
                        A_ti(a, b)
                    else:
                        A_qk(a, b)

                # score groups per (hc, par): nkb//2 of them; spread the
                # fillers evenly across them (Bresenham) so the PE always
                # has non-score work between groups while ACT drains exps.
                ngroups = 4 * 2 * (nkb // 2)
                nfill = len(fillers)
                gcount = 0
                npop = 0
                for hc in range(4):
                    pts[(tsl, hc)] = ptpool.tile(
                        [P, nkb, 2, 512], BF16, tag="pt", name=f"pt{tsl}_{hc}"
                    )
                    for par in range(2):
                        for g in range(nkb // 2):
                            B_sc(tsl, hc, par, g)
                            gcount += 1
                            while npop * ngroups < gcount * nfill:
                                emit_filler()
                                npop += 1
                    if hc >= 1:
                        B_av(tsl, hc - 1)
                pend3 = []
                for qt in range(tsl * 4, tsl * 4 + 4):
                    psy = AVq(tsl, 3, qt)
                    pend3.append((qt, psy))
                    if len(pend3) > 1:
                        q0, psy0 = pend3.pop(0)
                        NORMq(tsl, 3, q0, psy0)
                        if tsl == NQ - 1:
                            # yT column q0 is complete: project it out now
                            for eg in range(2):
                                O_ti(tsl, q0, eg)
                    emit_filler()
                while pend3:
                    q0, psy0 = pend3.pop(0)
                    NORMq(tsl, 3, q0, psy0)
                    if tsl == NQ - 1:
                        for eg in range(2):
                            O_ti(tsl, q0, eg)
                while fillers:
                    emit_filler()
    nc.compile()
    return nc


def build_ffn():
    """Launch 2: LN2 + GELU MLP + residual on a 1024-token slice.

    inputs : x2[1024,E] f32, w1[E,FF] bf16, w2[FF,E] bf16, b1[FF] f32
    output : out[1024,E] f32  (= x2 + gelu(LN(x2) @ w1 + b1) @ w2)

    ff1 is computed transposed (gT[f,t]) so the gelu output feeds the second
    matmul as lhsT without a transpose. W1 is loaded once; each chunk serves
    both 512-token halves. W2 arrives in chunks interleaved late (it is not
    needed until phase C), so startup DMA bandwidth goes to x and W1.
    """
    T = 1024
    nc = bacc.Bacc("TRN2", target_bir_lowering=False, debug=False, num_devices=8)
    x2_d = nc.dram_tensor("x2", [T, E], F32, kind="ExternalInput")
    w1_d = nc.dram_tensor("w1", [E, FF], BF16, kind="ExternalInput")
    w2_d = nc.dram_tensor("w2", [FF, E], BF16, kind="ExternalInput")
    b1_d = nc.dram_tensor("b1", [FF], F32, kind="ExternalInput")
    out_d = nc.dram_tensor("out", [T, E], F32, kind="ExternalOutput")

    NT = T // P  # 8 token tiles
    NF = FF // P  # 32 f chunks
    NFG = FF // 256  # 16 w1 dma chunks

    with tile.TileContext(nc) as tc:
        with (
            tc.tile_pool(name="consts", bufs=1) as consts,
            tc.tile_pool(name="state", bufs=1) as state,
            tc.tile_pool(name="w1p", bufs=6) as w1pool,
            tc.tile_pool(name="xin", bufs=4) as xin,
            tc.tile_pool(name="hp", bufs=2) as hpool,
            tc.tile_pool(name="outp", bufs=3) as outp,
            tc.tile_pool(name="small", bufs=6) as small,
            tc.tile_pool(name="psB", bufs=2, space="PSUM") as psB,
            tc.tile_pool(name="psC", bufs=4, space="PSUM") as psC,
        ):
            ident = consts.tile([P, P], BF16)
            eps_t = consts.tile([P, 1], F32)
            w2_sb = consts.tile([P, NF, E], BF16)
            b1_sb = consts.tile([P, NF], F32)
            h2T = state.tile([P, 8, T], BF16)  # [e_in, e_chunk, t]
            gT = state.tile([P, NF, T], BF16)  # [f_in, f_chunk, t]

            # preamble: x tiles first, then identity/eps/b1; w2 is emitted
            # in chunks interleaved into the ff1 loop below.
            xts = {}

            def issue_x(ti):
                xt = xin.tile([P, E], F32, tag="xt", name=f"xt{ti}")
                nc.sync.dma_start(xt[:], x2_d[ti * P : (ti + 1) * P, :])
                xts[ti] = xt

            for ti in range(4):
                issue_x(ti)
            make_identity(nc, ident)
            nc.vector.memset(eps_t[:], EPS)
            nc.sync.dma_start(b1_sb[:], b1_d.rearrange("(fo p) -> p fo", p=P))
            w1r = w1_d.rearrange("(eo p) f -> p eo f", p=P)
            w2r = w2_d.rearrange("(fo p) e -> p fo e", p=P)

            w1g_tiles = {}

            def issue_w1(fg):
                t = w1pool.tile([P, 8, 256], BF16, tag="w1")
                nc.sync.dma_start(t[:], w1r[:, :, fg * 256 : (fg + 1) * 256])
                w1g_tiles[fg] = t

            issue_w1(0)
            issue_w1(1)

            # PE warm-up: ramp the tensor-engine p-state while the first
            # x tiles and LN are still in flight.
            psW = psB.tile([P, P], BF16, tag="u", name="psW")
            for _ in range(75):
                nc.tensor.transpose(psW[:], ident[:], ident)

            def A_ti(to):
                xt = xts.pop(to)
                h2 = hpool.tile([P, E], BF16)
                _ln_tile(nc, small, xt[:], h2[:], eps_t)
                for g in range(2):
                    trp = psB.tile([P, 4, P], BF16, tag="u")
                    for j in range(4):
                        ec = g * 4 + j
                        nc.tensor.transpose(
                            trp[:, j, :], h2[:, ec * P : (ec + 1) * P], ident
                        )
                    nc.vector.tensor_copy(
                        h2T[:, g * 4 : (g + 1) * 4, to * P : (to + 1) * P],
                        trp[:],
                    )

            def ff1(fg, tsl, w1g):
                tofs = tsl * 512
                ps0 = psB.tile([P, 2, 512], F32, tag="u")
                for ec in range(8):
                    for j in range(2):
                        nc.tensor.matmul(
                            ps0[:, j, :],
                            lhsT=w1g[:, ec, j * P : (j + 1) * P],
                            rhs=h2T[:, ec, tofs : tofs + 512],
                            start=(ec == 0),
                            stop=(ec == 7),
                        )
                for j in range(2):
                    fc = fg * 2 + j
                    nc.scalar.activation(
                        gT[:, fc, tofs : tofs + 512],
                        ps0[:, j, :],
                        AF.Gelu,
                        bias=b1_sb[:, fc : fc + 1],
                    )

            def issue_w2(fg):
                if fg % 2 == 0 and fg // 2 < 8:
                    wsl = slice((fg // 2) * 4, (fg // 2) * 4 + 4)
                    nc.sync.dma_start(w2_sb[:, wsl, :], w2r[:, wsl, :])

            # ---- Phase A/B interleaved: LN+transpose first 4 tiles, then
            # ff1 on the first half while the second half's LN runs; the
            # first 4 W1 chunks stay resident so their tsl=1 pass follows.
            for to in range(4):
                A_ti(to)
            held = {}
            for fg in range(4):
                w1g = w1g_tiles.pop(fg)
                if fg + 2 < NFG:
                    issue_w1(fg + 2)
                issue_w2(fg)
                issue_x(4 + fg)
                ff1(fg, 0, w1g)
                A_ti(4 + fg)
                held[fg] = w1g
            for fg in range(4):
                ff1(fg, 1, held.pop(fg))
            for fg in range(4, NFG):
                w1g = w1g_tiles.pop(fg)
                if fg + 2 < NFG:
                    issue_w1(fg + 2)
                issue_w2(fg)
                ff1(fg, 0, w1g)
                ff1(fg, 1, w1g)

            # ---- Phase C: out = x2 + gT^T @ W2 ----
            for tb in range(NT):
                # residual tile first: its DMA hides behind the 64 matmuls
                xr = xin.tile([P, E], F32, tag="xt", name=f"xr{tb}")
                nc.sync.dma_start(xr[:], x2_d[tb * P : (tb + 1) * P, :])
                psO = psC.tile([P, 512], F32, tag="c")
                psP = psC.tile([P, 512], F32, tag="c")
                for fc in range(NF):
                    nc.tensor.matmul(
                        psO[:],
                        lhsT=gT[:, fc, tb * P : (tb + 1) * P],
                        rhs=w2_sb[:, fc, 0:512],
                        start=(fc == 0),
                        stop=(fc == NF - 1),
                    )
                    nc.tensor.matmul(
                        psP[:],
                        lhsT=gT[:, fc, tb * P : (tb + 1) * P],
                        rhs=w2_sb[:, fc, 512:1024],
                        start=(fc == 0),
                        stop=(fc == NF - 1),
                    )
                for eg, psX in ((0, psO), (1, psP)):
                    ot = outp.tile([P, 512], F32)
                    nc.vector.tensor_tensor(
                        out=ot[:],
                        in0=psX[:],
                        in1=xr[:, eg * 512 : (eg + 1) * 512],
                        op=OP.add,
                    )
                    nc.sync.dma_start(
                        out_d[tb * P : (tb + 1) * P, eg * 512 : (eg + 1) * 512],
                        ot[:],
                    )
    nc.compile()
    return nc


# ---------------------------------------------------------------------------
# Host orchestration
# ---------------------------------------------------------------------------


def _bf16(a):
    return np.ascontiguousarray(np.asarray(a, dtype=np.float32)).astype(BF16NP)


def _f32(a):
    return np.ascontiguousarray(np.asarray(a, dtype=np.float32))


def _tri01():
    kp = np.arange(P)[:, None]
    qf = np.arange(P)[None, :]
    t = (kp <= qf).astype(np.float32)
    return np.ascontiguousarray(np.stack([t, t], axis=1)).astype(BF16NP)


def kernel(
    x, Wq, bq, Wk, bk, Wv, bv, Wo, bo, g1, beta1, g2, beta2, W1, b1, W2, b2
):
    out, _ = _run(
        x, Wq, bq, Wk, bk, Wv, bv, Wo, bo, g1, beta1, g2, beta2, W1, b1, W2, b2
    )
    return out


def _run(
    x, Wq, bq, Wk, bk, Wv, bv, Wo, bo, g1, beta1, g2, beta2, W1, b1, W2, b2,
    trace=False,
):
    x = _f32(x)
    Wq, bq = _f32(Wq), _f32(bq)
    Wk, bk = _f32(Wk), _f32(bk)
    Wv, bv = _f32(Wv), _f32(bv)
    Wo, bo = _f32(Wo), _f32(bo)
    g1, beta1 = _f32(g1), _f32(beta1)
    g2, beta2 = _f32(g2), _f32(beta2)
    W1, b1 = _f32(W1), _f32(b1)
    W2, b2 = _f32(W2), _f32(b2)

    # Fold LN1 affine into the QKV projections: h = ln0*g1+beta1 =>
    # h@W + b == ln0@(g1[:,None]*W) + (beta1@W + b)
    Wq_e, bq_e = Wq * g1[:, None], beta1 @ Wq + bq
    Wk_e, bk_e = Wk * g1[:, None], beta1 @ Wk + bk
    Wv_e, bv_e = Wv * g1[:, None], beta1 @ Wv + bv
    # V-bias rides through the attention average (rows of attn sum to 1):
    # y = P@(v + bv) = P@v + bv  =>  fold bv@Wo into the residual bias.
    bo_e = bo + bv_e @ Wo
    # Fold LN2 affine into W1.
    W1_e, b1_e = W1 * g2[:, None], beta2 @ W1 + b1

    tri = _tri01()
    nc1 = build_attn()
    in_maps1 = []
    for c in range(8):
        b_, hh = c // 2, c % 2
        cs = 512 * hh
        in_maps1.append(
            {
                "x": x[b_],
                "wq": _bf16(Wq_e[:, cs : cs + 512]),
                "wk": _bf16(Wk_e[:, cs : cs + 512]),
                "wv": _bf16(Wv_e[:, cs : cs + 512]),
                "wo": _bf16(Wo[cs : cs + 512, :]),
                "bq": bq_e[cs : cs + 512],
                "bk": bk_e[cs : cs + 512],
                "tri": tri,
            }
        )
    res1 = run_bass_kernel_spmd(nc1, in_maps1, list(range(8)), trace=trace)
    x2 = x + bo_e[None, None, :]
    for c in range(8):
        x2[c // 2] += np.asarray(res1.results[c]["out"], dtype=np.float32)

    x2f = np.ascontiguousarray(x2.reshape(B * S, E), dtype=np.float32)
    w1b, w2b = _bf16(W1_e), _bf16(W2)
    nc2 = build_ffn()
    in_maps2 = [
        {
            "x2": x2f[c * 1024 : (c + 1) * 1024],
            "w1": w1b,
            "w2": w2b,
            "b1": b1_e,
        }
        for c in range(8)
    ]
    res2 = run_bass_kernel_spmd(nc2, in_maps2, list(range(8)), trace=trace)
    out = np.concatenate([res2.results[c]["out"] for c in range(8)], axis=0)
    out = out + b2[None, :]
    times = (res1.exec_time_ns, res2.exec_time_ns)
    return out.reshape(B, S, E).astype(np.float32), times
